# revision 1
# baseline (speedup 1.0000x reference)
"""Trainium2 Bass kernel for area-attention (YOLOv12 A2-style) module.

Raw-bass SPMD: 8 cores, 1 image each, no collectives. Engine split:
SP=DMA, PE=all matmuls (convs, dw-diag conv, attention with PE-array
row/col packing), ACT=softmax exp, DVE=evac/normalize/memset.
Software-pipelined per (area, head-group, q-tile) unit: exp(kc)
overlaps PV(kc-1)+scores(kc)+depthwise filler matmuls.
"""

import sys, os, functools

sys.path.insert(0, "/opt/trn_rl_repo")

import numpy as np
import ml_dtypes

import concourse.bass as bass
import concourse.mybir as mybir
from concourse.bass_utils import run_bass_kernel_spmd

BF16NP = ml_dtypes.bfloat16
F32 = mybir.dt.float32
BF = mybir.dt.bfloat16

C = 256
HH = 64
WW = 64
NTOK = HH * WW          # 4096
AREA = 4
NA = NTOK // AREA       # 1024
D = 32                  # head dim
SCALE = float(D) ** -0.5
PADW = WW + 4           # 68
PADH = HH + 4           # 68
NPAD = PADW * PADH      # 4624
AF = mybir.ActivationFunctionType

UNITS = [(a, hg, qt) for a in range(AREA) for hg in range(2) for qt in range(2)]
# dw output tile needed by proj of area a: (ct, 2a+qt)
DW_TILES = [(hg, 2 * a + qt) for a in range(AREA) for hg in range(2) for qt in range(2)]
N_IN_DMA = 12


def _build_nc():
    import contextlib
    nc = bass.Bass()

    x_d = nc.declare_dram_parameter("x", [C, NTOK], BF, isOutput=False)
    wqkT_d = nc.declare_dram_parameter("wqkT", [C, 2 * C], BF, isOutput=False)
    wvT_d = nc.declare_dram_parameter("wvT", [C, C], BF, isOutput=False)
    wprojT_d = nc.declare_dram_parameter("wprojT", [C, C], BF, isOutput=False)
    dwdiag_d = nc.declare_dram_parameter("dwdiag", [2, 128, 25 * 128], BF, isOutput=False)
    ones_d = nc.declare_dram_parameter("ones", [128, 512], BF, isOutput=False)
    brows_d = nc.declare_dram_parameter("brows", [1, 1024], BF, isOutput=False)
    out_d = nc.declare_dram_parameter("out", [C, NTOK], F32, isOutput=True)

    ctx = contextlib.ExitStack()
    _names = [0]

    def sb(shape, dtype, nm=None):
        _names[0] += 1
        return ctx.enter_context(nc.sbuf_tensor(f"sb{_names[0]}", shape, dtype))[:, :]

    def ps(shape):
        _names[0] += 1
        return ctx.enter_context(nc.psum_tensor(f"ps{_names[0]}", shape, F32))[:, :]

    x_sb = [sb([128, NTOK], BF) for _ in range(2)]
    qk_sb = [sb([128, NTOK], BF) for _ in range(4)]
    vT_sb = sb([128, 32 * 256], BF)
    vpad_sb = [sb([128, NPAD], BF) for _ in range(2)]
    pp_sb = [sb([128, NTOK], BF) for _ in range(2)]
    oT_sb = [sb([128, NTOK], BF) for _ in range(2)]
    z_sb = [sb([128, NTOK], BF) for _ in range(2)]
    wqkT_sb = [sb([128, 2 * C], BF) for _ in range(2)]
    wvT_sb = [sb([128, C], BF) for _ in range(2)]
    wprojT_sb = [sb([128, C], BF) for _ in range(2)]
    dw_sb = [sb([128, 25 * 128], BF) for _ in range(2)]
    ones_sb = sb([128, 512], BF)
    brows_sb = sb([1, 1024], BF)
    rcinv_sb = sb([128, 512], BF)
    es_sb = [sb([128, 2048], BF) for _ in range(2)]  # [buf]
    ucp_sb = sb([128, 512], F32)
    out_sb = [sb([128, 512], F32) for _ in range(16)]

    sc_ps = ps([128, 2048])   # 4 banks
    u_ps = ps([128, 512])
    r_ps = ps([128, 512])
    rb_ps = r_ps  # shared bank: recip reads r before bcast overwrites (sem-ordered)
    conv_ps = [ps([128, 512]) for _ in range(2)]

    vpad3 = [vp.rearrange("p (h w) -> p h w", h=PADH) for vp in vpad_sb]

    marks = {}  # name -> counter value (producer engine count at completion)

    # conv-bank users in PE order: list of keys; user i uses bank i%2 and
    # must WAR-wait on evac of user i-2 (marks['ev_'+key]).
    conv_users = []
    for m in range(4):
        for n in range(8):
            conv_users.append(f"qk_{m}_{n}")
    for t in range(32):
        conv_users.append(f"vt_{t}")
    for m in range(2):
        for n in range(8):
            conv_users.append(f"v_{m}_{n}")
    # dw tile u interleaved within unit u; proj tiles after each area
    seq = []
    for u, (a, hg, qt) in enumerate(UNITS):
        seq.append(f"dw_{u}")
        if hg == 1 and qt == 1:
            for nt in range(2):
                for m in range(2):
                    seq.append(f"pj_{2 * a + nt}_{m}")
    conv_users.extend(seq)
    user_bank = {k: i % 2 for i, k in enumerate(conv_users)}
    user_prev = {k: (conv_users[i - 2] if i >= 2 else None)
                 for i, k in enumerate(conv_users)}

    class Eng:
        """Counts instructions; in real mode also emits via `fns`."""

        def __init__(self, name, sem_name):
            self.name = name
            self.sem_name = sem_name
            self.n = 0

        def bump(self, binst, real, sems):
            self.n += 1
            if real:
                binst.then_inc(sems[self.sem_name], 1)

    def program(real, engines, sems):
        pe, act, dve, sp = engines["pe"], engines["act"], engines["dve"], engines["sp"]

        def w(eng_handle, sem_name, val):
            # standalone wait_ge; no count
            if real and val is not None and val > 0:
                eng_handle.wait_ge(sems[sem_name], val)

        def mark(name, eng):
            if not real:
                marks[name] = eng.n

        def get(name):
            return marks.get(name, 0)

        # ---------------- SP ----------------
        def sp_prog(h):
            n_dma = 0
            if real:
                for i in range(2):
                    h.dma_start(out=wqkT_sb[i], in_=wqkT_d[i * 128:(i + 1) * 128, :]).then_inc(sems["dma_i"], 16)
                    h.dma_start(out=wvT_sb[i], in_=wvT_d[i * 128:(i + 1) * 128, :]).then_inc(sems["dma_i"], 16)
                    h.dma_start(out=wprojT_sb[i], in_=wprojT_d[i * 128:(i + 1) * 128, :]).then_inc(sems["dma_i"], 16)
                    h.dma_start(out=dw_sb[i], in_=dwdiag_d[i, :, :]).then_inc(sems["dma_i"], 16)
                h.dma_start(out=ones_sb, in_=ones_d[:, :]).then_inc(sems["dma_i"], 16)
                h.dma_start(out=brows_sb, in_=brows_d[:, :]).then_inc(sems["dma_i"], 16)
                for i in range(2):
                    h.dma_start(out=x_sb[i], in_=x_d[i * 128:(i + 1) * 128, :]).then_inc(sems["dma_i"], 16)
            n_dma = 12
            assert n_dma == N_IN_DMA
            # out DMAs
            for k in range(8):            # token tile n
                for m in range(2):
                    idx = k * 2 + m
                    if real:
                        h.wait_ge(sems["dve_s"], marks[f"po_{k}_{m}"])
                        h.dma_start(out=out_d[m * 128:(m + 1) * 128, k * 512:(k + 1) * 512],
                                    in_=out_sb[idx]).then_inc(sems["dma_o"], 16)
            if real:
                h.wait_ge(sems["dma_o"], 16 * 16)

        # ---------------- PE ----------------
        def pe_prog(h):
            def mm(out, lhsT, rhs, start, stop, tp=None):
                if real:
                    i = nc.tensor.matmul(out, lhsT, rhs, start=start, stop=stop,
                                         tile_position=tp, skip_group_check=True)
                    pe.bump(i, real, sems)
                else:
                    pe.n += 1

            def conv_war(key):
                prev = user_prev[key]
                if prev is not None:
                    w(h, "dve_s", get(f"ev_{prev}"))

            w(h, "dma_i", N_IN_DMA * 16)
            # qk GEMM
            for m in range(4):
                for n in range(8):
                    key = f"qk_{m}_{n}"
                    b = user_bank[key]
                    conv_war(key)
                    mm(conv_ps[b], wqkT_sb[0][:, m * 128:(m + 1) * 128],
                       x_sb[0][:, n * 512:(n + 1) * 512], True, False)
                    mm(conv_ps[b], wqkT_sb[1][:, m * 128:(m + 1) * 128],
                       x_sb[1][:, n * 512:(n + 1) * 512], False, False)
                    mm(conv_ps[b], brows_sb[0:1, m * 128:(m + 1) * 128],
                       ones_sb[0:1, 0:512], False, True)
                    mark(key, pe)
            # vT GEMM
            for t in range(32):
                key = f"vt_{t}"
                b = user_bank[key]
                conv_war(key)
                mm(conv_ps[b][:, 0:256], x_sb[0][:, t * 128:(t + 1) * 128], wvT_sb[0], True, False)
                mm(conv_ps[b][:, 0:256], x_sb[1][:, t * 128:(t + 1) * 128], wvT_sb[1], False, True)
                mark(key, pe)
            # v GEMM
            for m in range(2):
                for n in range(8):
                    key = f"v_{m}_{n}"
                    b = user_bank[key]
                    conv_war(key)
                    mm(conv_ps[b], wvT_sb[0][:, m * 128:(m + 1) * 128],
                       x_sb[0][:, n * 512:(n + 1) * 512], True, False)
                    mm(conv_ps[b], wvT_sb[1][:, m * 128:(m + 1) * 128],
                       x_sb[1][:, n * 512:(n + 1) * 512], False, True)
                    mark(key, pe)
            # wait all startup evacs (qk ready for scores, vT for PV, vpad for dw)
            w(h, "dve_s", get("startup_evac"))

            for u, (a, hg, qt) in enumerate(UNITS):
                qb = a * NA + qt * 512

                def scores(kc):
                    kb = a * NA + kc * 128
                    for j in range(4):
                        mm(sc_ps[:, j * 512:(j + 1) * 512],
                           qk_sb[2 + hg][32 * j:32 * j + 32, kb:kb + 128],
                           qk_sb[hg][32 * j:32 * j + 32, qb:qb + 512],
                           True, True, tp=(32 * j, 0))

                def pv(kc):
                    tvt = a * 8 + kc
                    ebuf = es_sb[kc % 2]
                    for j in range(4):
                        esj = ebuf[:, j * 512:(j + 1) * 512]
                        mm(u_ps[32 * j:32 * j + 32, :],
                           vT_sb[:, tvt * 256 + hg * 128 + 32 * j:
                                 tvt * 256 + hg * 128 + 32 * j + 32],
                           esj, kc == 0, kc == 7, tp=(0, 32 * j))
                    for j in range(4):
                        esj = ebuf[:, j * 512:(j + 1) * 512]
                        mm(r_ps[32 * j:32 * j + 1, :], ones_sb[:, 0:1], esj,
                           kc == 0, kc == 7, tp=(0, 32 * j))

                # dw filler chunks for dw tile u
                ct, dn = DW_TILES[u]
                dwkey = f"dw_{u}"
                dwb = user_bank[dwkey]

                def dw_chunk(ci):
                    taps = range((25 * ci) // 8, (25 * (ci + 1)) // 8)
                    for tap in taps:
                        if tap == 0:
                            conv_war(dwkey)
                        dy, dx = divmod(tap, 5)
                        mm(conv_ps[dwb], dw_sb[ct][:, tap * 128:(tap + 1) * 128],
                           vpad3[ct][:, 8 * dn + dy:8 * dn + dy + 8, dx:dx + WW],
                           tap == 0, tap == 24)
                    if taps and max(taps) == 24:
                        mark(dwkey, pe)

                # unit prologue: WAR on U/r/rb banks vs previous unit's DVE reads
                if u > 0:
                    w(h, "dve_s", get(f"unit_dve_{u - 1}"))
                scores(0)
                mark(f"grp_{u}_0", pe)
                for kc in range(1, 9):
                    dw_chunk(kc - 1)
                    w(h, "act_s", get(f"exp_{u}_{kc - 1}"))
                    pv(kc - 1)
                    if kc < 8:
                        scores(kc)
                        mark(f"grp_{u}_{kc}", pe)
                mark(f"unitpv_{u}", pe)
                # rb broadcast (needs recip on DVE)
                w(h, "dve_s", get(f"recip_{u}"))
                for j in range(4):
                    mm(rb_ps[32 * j:32 * j + 32, :],
                       ones_sb[32 * j:32 * j + 1, 0:32],
                       rcinv_sb[32 * j:32 * j + 1, :],
                       True, True, tp=(32 * j, 32 * j))
                mark(f"rb_{u}", pe)

                if hg == 1 and qt == 1:
                    # proj for area a
                    w(h, "dve_s", get(f"z_{a}"))
                    for nt in range(2):
                        k = 2 * a + nt
                        for m in range(2):
                            key = f"pj_{k}_{m}"
                            b = user_bank[key]
                            conv_war(key)
                            mm(conv_ps[b], wprojT_sb[0][:, m * 128:(m + 1) * 128],
                               z_sb[0][:, k * 512:(k + 1) * 512], True, False)
                            mm(conv_ps[b], wprojT_sb[1][:, m * 128:(m + 1) * 128],
                               z_sb[1][:, k * 512:(k + 1) * 512], False, False)
                            mm(conv_ps[b], brows_sb[0:1, 512 + m * 128:512 + (m + 1) * 128],
                               ones_sb[0:1, 0:512], False, True)
                            mark(key, pe)

        # ---------------- ACT ----------------
        def act_prog(h):
            def ex(out, in_):
                if real:
                    i = nc.scalar.activation(out, in_, AF.Exp)
                    act.bump(i, real, sems)
                else:
                    act.n += 1

            for u, (a, hg, qt) in enumerate(UNITS):
                for kc in range(8):
                    w(h, "pe_s", get(f"grp_{u}_{kc}"))
                    ex(es_sb[kc % 2], sc_ps)
                    mark(f"exp_{u}_{kc}", act)

        # ---------------- DVE ----------------
        def dve_prog(h):
            def selfwait():
                if real:
                    h.wait_ge(sems["dve_s"], dve.n)

            def op(fn, *args, **kw):
                if real:
                    i = fn(*args, **kw)
                    dve.bump(i, real, sems)
                else:
                    dve.n += 1

            op(nc.vector.memset, vpad_sb[0], 0.0)
            op(nc.vector.memset, vpad_sb[1], 0.0)
            op(nc.vector.memset, r_ps, 1.0)

            for m in range(4):
                for n in range(8):
                    key = f"qk_{m}_{n}"
                    w(h, "pe_s", get(key))
                    op(nc.vector.tensor_copy,
                       qk_sb[m][:, n * 512:(n + 1) * 512], conv_ps[user_bank[key]])
                    mark(f"ev_{key}", dve)
            for t in range(32):
                key = f"vt_{t}"
                w(h, "pe_s", get(key))
                op(nc.vector.tensor_copy,
                   vT_sb[:, t * 256:(t + 1) * 256], conv_ps[user_bank[key]][:, 0:256])
                mark(f"ev_{key}", dve)
            selfwait()
            for m in range(2):
                for n in range(8):
                    key = f"v_{m}_{n}"
                    w(h, "pe_s", get(key))
                    op(nc.vector.tensor_copy,
                       vpad3[m][:, 2 + 8 * n:2 + 8 * n + 8, 2:2 + WW],
                       conv_ps[user_bank[key]].rearrange("p (r w) -> p r w", r=8))
                    mark(f"ev_{key}", dve)
            mark("startup_evac", dve)

            for u, (a, hg, qt) in enumerate(UNITS):
                qb = a * NA + qt * 512
                w(h, "pe_s", get(f"unitpv_{u}"))
                if real:
                    lp = nc.allow_low_precision("softmax denom to bf16")
                    lp.__enter__()
                selfwait()
                op(nc.vector.reciprocal, rcinv_sb, r_ps)
                if real:
                    lp.__exit__(None, None, None)
                mark(f"recip_{u}", dve)
                op(nc.vector.tensor_copy, ucp_sb, u_ps)
                w(h, "pe_s", get(f"rb_{u}"))
                selfwait()
                op(nc.vector.tensor_mul,
                   oT_sb[hg][:, qb:qb + 512], ucp_sb, rb_ps)
                mark(f"unit_dve_{u}", dve)

                # dw evac for tile u
                ct, dn = DW_TILES[u]
                dwkey = f"dw_{u}"
                w(h, "pe_s", get(dwkey))
                op(nc.vector.tensor_copy,
                   pp_sb[ct][:, dn * 512:(dn + 1) * 512], conv_ps[user_bank[dwkey]])
                mark(f"ev_{dwkey}", dve)

                if hg == 1 and qt == 1:
                    # z = o + pp for area a
                    selfwait()
                    for cti in range(2):
                        op(nc.vector.tensor_add,
                           z_sb[cti][:, a * NA:(a + 1) * NA],
                           oT_sb[cti][:, a * NA:(a + 1) * NA],
                           pp_sb[cti][:, a * NA:(a + 1) * NA])
                    mark(f"z_{a}", dve)
                    # proj evacs
                    for nt in range(2):
                        k = 2 * a + nt
                        for m in range(2):
                            key = f"pj_{k}_{m}"
                            idx = k * 2 + m
                            w(h, "pe_s", get(key))
                            op(nc.vector.tensor_copy, out_sb[idx],
                               conv_ps[user_bank[key]])
                            mark(f"ev_{key}", dve)
                            mark(f"po_{k}_{m}", dve)

        if real:
            with nc.Block() as block, \
                 nc.semaphore("dma_i") as s_dma_i, \
                 nc.semaphore("dma_o") as s_dma_o, \
                 nc.semaphore("pe_s") as s_pe, \
                 nc.semaphore("act_s") as s_act, \
                 nc.semaphore("dve_s") as s_dve:
                sems.update({"dma_i": s_dma_i, "dma_o": s_dma_o,
                             "pe_s": s_pe, "act_s": s_act, "dve_s": s_dve})

                @block.sync
                def _(sync):
                    sp_prog(sync)

                @block.tensor
                def _(tensor):
                    pe_prog(tensor)

                @block.scalar
                def _(scalar):
                    act_prog(scalar)

                @block.vector
                def _(vector):
                    dve_prog(vector)
        else:
            class H:  # dry handle
                def wait_ge(self, *a, **k):
                    pass

                def dma_start(self, *a, **k):
                    class R:
                        def then_inc(self, *a, **k):
                            return self
                    return R()
            hh = H()
            sp_prog(hh)
            pe_prog(hh)
            act_prog(hh)
            dve_prog(hh)

    engines = {"pe": Eng("pe", "pe_s"), "act": Eng("act", "act_s"),
               "dve": Eng("dve", "dve_s"), "sp": Eng("sp", "dma_i")}
    sems = {}
    program(False, engines, sems)          # dry: fill marks
    engines = {"pe": Eng("pe", "pe_s"), "act": Eng("act", "act_s"),
               "dve": Eng("dve", "dve_s"), "sp": Eng("sp", "dma_i")}
    program(True, engines, sems)           # real emission
    return nc


@functools.lru_cache(maxsize=1)
def _get_nc():
    return _build_nc()


def _prep_host(inputs):
    x = np.asarray(inputs["x"], np.float32)            # [8, 256, 64, 64]
    w_qk = np.asarray(inputs["w_qk"], np.float32)      # [512, 256]
    s_qk = np.asarray(inputs["s_qk"], np.float32)
    b_qk = np.asarray(inputs["b_qk"], np.float32)
    w_v = np.asarray(inputs["w_v"], np.float32)
    s_v = np.asarray(inputs["s_v"], np.float32)
    b_v = np.asarray(inputs["b_v"], np.float32)
    w_pe = np.asarray(inputs["w_pe"], np.float32)      # [256, 1, 5, 5]
    s_pe = np.asarray(inputs["s_pe"], np.float32)
    b_pe = np.asarray(inputs["b_pe"], np.float32)
    w_proj = np.asarray(inputs["w_proj"], np.float32)
    s_proj = np.asarray(inputs["s_proj"], np.float32)
    b_proj = np.asarray(inputs["b_proj"], np.float32)

    # fold BN scales into weights; fold 1/sqrt(d) into q weights+bias
    w_qk_eff = w_qk * s_qk[:, None]
    b_qk_eff = b_qk * s_qk  # BN affine: y = s*(Wx) + b ... b is already the bias
    # NB: reference _conv1x1 computes  y = (Wx)*s + b, so bias is NOT scaled by s.
    b_qk_eff = b_qk.copy()
    w_qk_eff[:C] *= SCALE
    b_qk_eff[:C] *= SCALE

    w_v_eff = w_v * s_v[:, None]
    w_proj_eff = w_proj * s_proj[:, None]

    wpe = w_pe.reshape(C, 25)                          # [c, tap]
    wpe_eff = wpe * s_pe[:, None]

    # constants folded through attention/depthwise into proj bias:
    # o gets +b_v exactly (softmax rows sum to 1);
    # pp = s_pe*dw(v_nb) + s_pe*b_v*sum_taps(w_pe) + b_pe
    kappa = b_v + s_pe * b_v * wpe.sum(1) + b_pe       # [256]
    b_proj_eff = b_proj + w_proj_eff @ kappa

    dwdiag = np.zeros((2, 128, 25 * 128), np.float32)
    for ct in range(2):
        for tap in range(25):
            idx = np.arange(128)
            dwdiag[ct, idx, tap * 128 + idx] = wpe_eff[ct * 128 + idx, tap]

    common = {
        "wqkT": np.ascontiguousarray(w_qk_eff.T).astype(BF16NP),

        "wvT": np.ascontiguousarray(w_v_eff.T).astype(BF16NP),
        "wprojT": np.ascontiguousarray(w_proj_eff.T).astype(BF16NP),
        "brows": np.concatenate([b_qk_eff, b_proj_eff, np.zeros(256, np.float32)]
                                ).reshape(1, 1024).astype(BF16NP),
        "dwdiag": dwdiag.astype(BF16NP),
        "ones": np.ones((128, 512), BF16NP),
    }
    in_maps = []
    for i in range(8):
        m = dict(common)
        m["x"] = np.ascontiguousarray(x[i].reshape(C, NTOK)).astype(BF16NP)
        in_maps.append(m)
    return in_maps


def kernel(**inputs):
    nc = _get_nc()
    in_maps = _prep_host(inputs)
    res = run_bass_kernel_spmd(nc, in_maps, core_ids=list(range(8)))
    outs = [res.results[i]["out"].reshape(C, HH, WW) for i in range(8)]
    return np.stack(outs, 0).astype(np.float32)


if __name__ == "__main__":
    rng = np.random.default_rng(0)
    fake = {
        "x": rng.standard_normal((8, C, HH, WW), np.float32),
        "w_qk": rng.standard_normal((2 * C, C), np.float32) * 0.05,
        "s_qk": np.ones(2 * C, np.float32),
        "b_qk": rng.standard_normal(2 * C).astype(np.float32) * 0.01,
        "w_v": rng.standard_normal((C, C), np.float32) * 0.05,
        "s_v": np.ones(C, np.float32),
        "b_v": rng.standard_normal(C).astype(np.float32) * 0.01,
        "w_pe": rng.standard_normal((C, 1, 5, 5), np.float32) * 0.05,
        "s_pe": np.ones(C, np.float32),
        "b_pe": rng.standard_normal(C).astype(np.float32) * 0.01,
        "w_proj": rng.standard_normal((C, C), np.float32) * 0.05,
        "s_proj": np.ones(C, np.float32),
        "b_proj": rng.standard_normal(C).astype(np.float32) * 0.01,
    }
    out = kernel(**fake)
    print("out", out.shape, out.dtype, float(np.abs(out).mean()))



# revision 18
# speedup vs baseline: 1.2400x; 1.2400x over previous
"""Trainium2 Bass kernel for area-attention (YOLOv12 A2-style) module.

Raw-bass SPMD: 8 cores, 1 image each, no collectives.

v2 design (vs baseline):
- softmax exp split between ACT (native Exp, 10/16 chunks) and DVE
  (Schraudolph bit-trick exp: i16 = rne(t*128 + B) bitcast to bf16,
  6/16 chunks). Scores computed as t = log2(e)*s by folding log2e into
  q weights; ACT uses scale=ln2.
- score PSUM double-buffered ([128,1024] x2) so the two exp engines
  overlap; chunk = (k-block 128) x (4 heads x 256 q).
- depthwise 5x5 conv as 10 rounds of 4-way col-tiled matmuls with
  4 taps stacked in the contract dim; shifted v copies built by
  SBUF->SBUF DMA.
- biases folded into DVE evacuations (tensor_scalar add), no rank-1
  bias matmuls; semaphore inc only on the last matmul of each
  concurrent group.
"""

import sys, os, functools

sys.path.insert(0, "/opt/trn_rl_repo")

import numpy as np
import ml_dtypes

import concourse.bass as bass
import concourse.mybir as mybir
from concourse.bass_utils import run_bass_kernel_spmd

BF16NP = ml_dtypes.bfloat16
F32 = mybir.dt.float32
BF = mybir.dt.bfloat16
I16 = mybir.dt.int16
AF = mybir.ActivationFunctionType
ALU = mybir.AluOpType

C = 256
HH = 64
WW = 64
NTOK = HH * WW          # 4096
AREA = 4
NA = NTOK // AREA       # 1024
D = 32                  # head dim
SCALE = float(D) ** -0.5
LOG2E = float(np.log2(np.e))
LN2 = float(np.log(2.0))
PADW = WW + 4           # 68
PADH = HH + 4           # 68
NPAD = PADW * PADH      # 4624
SCH_B = 16248.636       # Schraudolph bias, mean-centered vs exact exp

UNITS = [(a, hg, qt) for a in range(AREA) for hg in range(2) for qt in range(2)]
DVE_CHUNKS = (1, 3, 6, 9, 11, 14)   # chunks (of 16/unit) exp'd on DVE
N_IN_DMA = 18
KVAR = os.environ.get("KVAR", "")   # bisect switches: "nodw", "noi16"
if "noi16" in KVAR:
    DVE_CHUNKS = ()

# dw round r: r even -> dy0=0 taps dy=0..3 (blocks 0-3); r odd -> dy0=4
# tap dy=4 (block 0 only); dx = r//2.
DW_ROUNDS = 10
# dw round -> pipeline slot (0..15) for interleaving
DW_SLOT = [(r * 16) // DW_ROUNDS for r in range(DW_ROUNDS)]


def _build_nc():
    import contextlib
    nc = bass.Bass()

    x_d = nc.declare_dram_parameter("x", [C, NTOK], BF, isOutput=False)
    wqkT_d = nc.declare_dram_parameter("wqkT", [C, 2 * C], BF, isOutput=False)
    wvT_d = nc.declare_dram_parameter("wvT", [C, C], BF, isOutput=False)
    wprojT_d = nc.declare_dram_parameter("wprojT", [C, C], BF, isOutput=False)
    dwstk_d = nc.declare_dram_parameter("dwstk", [8, 128, DW_ROUNDS * 32], BF, isOutput=False)
    ones_d = nc.declare_dram_parameter("ones", [128, 512], BF, isOutput=False)
    bias_d = nc.declare_dram_parameter("bias", [128, 8], F32, isOutput=False)
    out_d = nc.declare_dram_parameter("out", [C, NTOK], F32, isOutput=True)

    ctx = contextlib.ExitStack()
    _names = [0]

    def sb(shape, dtype):
        _names[0] += 1
        return ctx.enter_context(nc.sbuf_tensor(f"sb{_names[0]}", shape, dtype))[:, :]

    def ps(shape):
        _names[0] += 1
        return ctx.enter_context(nc.psum_tensor(f"ps{_names[0]}", shape, F32))[:, :]

    x_sb = [sb([128, NTOK], BF) for _ in range(2)]
    qk_sb = [sb([128, NTOK], BF) for _ in range(4)]
    vT_sb = sb([128, 32 * 256], BF)
    vpad_sb = [sb([128, NPAD], BF) for _ in range(2)]
    stk_sb = [sb([128, NPAD], BF) for _ in range(8)]
    pp_sb = [sb([128, NA], BF) for _ in range(2)]
    oT_sb = [sb([128, NA], BF) for _ in range(2)]
    z_sb = [sb([128, NA], BF) for _ in range(2)]
    es_sb = [sb([128, 1024], BF) for _ in range(2)]
    rcinv_sb = sb([128, 512], BF)
    ucp_sb = sb([128, 512], F32)
    out_sb = [sb([128, 512], F32) for _ in range(4)]
    wqkT_sb = [sb([128, 2 * C], BF) for _ in range(2)]
    wvT_sb = [sb([128, C], BF) for _ in range(2)]
    wprojT_sb = [sb([128, C], BF) for _ in range(2)]
    dwstk_sb = [sb([128, DW_ROUNDS * 32], BF) for _ in range(8)]
    ones_sb = sb([128, 512], BF)
    bias_sb = sb([128, 8], F32)

    sc_ps = [ps([128, 1024]) for _ in range(2)]
    u_ps = ps([128, 512])
    r_ps = ps([128, 512])       # also holds rb broadcast after recip
    conv_ps = [ps([128, 512]) for _ in range(2)]

    vpad3 = [vp.rearrange("p (h w) -> p h w", h=PADH) for vp in vpad_sb]
    stk3 = [st.rearrange("p (h w) -> p h w", h=PADH) for st in stk_sb]
    es_i16 = [e.bitcast(I16) for e in es_sb]

    # conv-bank users in PE order; user i uses bank i%2 and WAR-waits on
    # evac of user i-2.
    conv_users = []
    for m in range(4):
        for n in range(8):
            conv_users.append(f"qk_{m}_{n}")
    for t in range(32):
        conv_users.append(f"vt_{t}")
    for m in range(2):
        for n in range(8):
            conv_users.append(f"v_{m}_{n}")
    for u, (a, hg, qt) in enumerate(UNITS):
        conv_users.append(f"dw_{u}")
        if hg == 1 and qt == 1:
            for nt in range(2):
                for m in range(2):
                    conv_users.append(f"pj_{a}_{nt}_{m}")
    user_bank = {k: i % 2 for i, k in enumerate(conv_users)}
    user_prev = {k: (conv_users[i - 2] if i >= 2 else None)
                 for i, k in enumerate(conv_users)}

    marks = {}  # name -> (sem_name, count)

    class Eng:
        def __init__(self, name, sem_name):
            self.name = name
            self.sem_name = sem_name
            self.n = 0          # number of semaphore increments so far

        def bump(self, binst, real, sems):
            self.n += 1
            if real:
                binst.then_inc(sems[self.sem_name], 1)

    def program(real, engines, sems):
        pe, act, dve, sp = engines["pe"], engines["act"], engines["dve"], engines["sp"]

        def wmark(h, name):
            if real:
                sem_name, val = marks[name]
                if val > 0:
                    h.wait_ge(sems[sem_name], val)

        def wraw(h, sem_name, val):
            if real and val > 0:
                h.wait_ge(sems[sem_name], val)

        def mark(name, eng):
            if not real:
                marks[name] = (eng.sem_name, eng.n)

        # ---------------- SP ----------------
        def sp_prog(h):
            if real:
                for i in range(2):
                    h.dma_start(out=wqkT_sb[i], in_=wqkT_d[i * 128:(i + 1) * 128, :]).then_inc(sems["dma_i"], 16)
                    h.dma_start(out=wvT_sb[i], in_=wvT_d[i * 128:(i + 1) * 128, :]).then_inc(sems["dma_i"], 16)
                    h.dma_start(out=wprojT_sb[i], in_=wprojT_d[i * 128:(i + 1) * 128, :]).then_inc(sems["dma_i"], 16)
                for g in range(8):
                    h.dma_start(out=dwstk_sb[g], in_=dwstk_d[g, :, :]).then_inc(sems["dma_i"], 16)
                h.dma_start(out=ones_sb, in_=ones_d[:, :]).then_inc(sems["dma_i"], 16)
                h.dma_start(out=bias_sb, in_=bias_d[:, :]).then_inc(sems["dma_i"], 16)
                for i in range(2):
                    h.dma_start(out=x_sb[i], in_=x_d[i * 128:(i + 1) * 128, :]).then_inc(sems["dma_i"], 16)
            # stack-building DMAs (SBUF->SBUF shifted copies); per-half
            # semaphore so the wait covers ALL of that sem's increments
            # (DMA completions are unordered across transfers).
            for half in range(2):
                if "nodw" in KVAR:
                    break
                nk = 0
                if real:
                    wmark(h, "memsets_done")
                    wmark(h, f"ev_vhalf_{half}")
                for g in range(4 * half, 4 * half + 4):
                    rows = slice((g % 4) * 32, (g % 4) * 32 + 32)
                    for t in range(4):
                        if real:
                            h.dma_start(
                                out=stk_sb[g][32 * t:32 * t + 32, 0:NPAD - 68 * t],
                                in_=vpad_sb[half][rows, 68 * t:NPAD],
                            ).then_inc(sems[f"dma_k{half}"], 16)
                        nk += 16
                if not real:
                    marks[f"stk_half_{half}"] = (f"dma_k{half}", nk)
            # output DMAs; per-slot semaphore (same reason)
            for a in range(AREA):
                for nt in range(2):
                    for m in range(2):
                        if real:
                            wmark(h, f"po_{a}_{nt}_{m}")
                            h.dma_start(
                                out=out_d[m * 128:(m + 1) * 128,
                                          (2 * a + nt) * 512:(2 * a + nt + 1) * 512],
                                in_=out_sb[nt * 2 + m]).then_inc(sems[f"dma_o{nt * 2 + m}"], 16)
            if real:
                for s in range(4):
                    h.wait_ge(sems[f"dma_o{s}"], 16 * AREA)

        # ---------------- PE ----------------
        def pe_prog(h):
            def mm(out, lhsT, rhs, start, stop, tp=None, inc=False):
                if real:
                    i = nc.tensor.matmul(out, lhsT, rhs, start=start, stop=stop,
                                         tile_position=tp, skip_group_check=True)
                    if inc:
                        pe.bump(i, real, sems)
                elif inc:
                    pe.n += 1

            def conv_war(key):
                prev = user_prev[key]
                if prev is not None:
                    wmark(h, f"ev_{prev}")

            wraw(h, "dma_i", N_IN_DMA * 16)
            # qk GEMM
            for m in range(4):
                for n in range(8):
                    key = f"qk_{m}_{n}"
                    b = user_bank[key]
                    conv_war(key)
                    mm(conv_ps[b], wqkT_sb[0][:, m * 128:(m + 1) * 128],
                       x_sb[0][:, n * 512:(n + 1) * 512], True, False)
                    mm(conv_ps[b], wqkT_sb[1][:, m * 128:(m + 1) * 128],
                       x_sb[1][:, n * 512:(n + 1) * 512], False, True, inc=True)
                    mark(key, pe)
            # vT GEMM ([tok, chan] tiles)
            for t in range(32):
                key = f"vt_{t}"
                b = user_bank[key]
                conv_war(key)
                mm(conv_ps[b][:, 0:256], x_sb[0][:, t * 128:(t + 1) * 128], wvT_sb[0], True, False)
                mm(conv_ps[b][:, 0:256], x_sb[1][:, t * 128:(t + 1) * 128], wvT_sb[1], False, True, inc=True)
                mark(key, pe)
            # v GEMM ([chan, tok] for dw)
            for m in range(2):
                for n in range(8):
                    key = f"v_{m}_{n}"
                    b = user_bank[key]
                    conv_war(key)
                    mm(conv_ps[b], wvT_sb[0][:, m * 128:(m + 1) * 128],
                       x_sb[0][:, n * 512:(n + 1) * 512], True, False)
                    mm(conv_ps[b], wvT_sb[1][:, m * 128:(m + 1) * 128],
                       x_sb[1][:, n * 512:(n + 1) * 512], False, True, inc=True)
                    mark(key, pe)

            for u, (a, hg, qt) in enumerate(UNITS):
                if "nounits" in KVAR:
                    break
                qbase = a * NA + qt * 512
                ct, dn = hg, 2 * a + qt
                dwkey = f"dw_{u}"
                dwb = user_bank[dwkey]

                def scores(c):
                    kb = a * NA + (c >> 1) * 128
                    qcol = qbase + (c & 1) * 256
                    for j in range(4):
                        mm(sc_ps[c & 1][:, j * 256:(j + 1) * 256],
                           qk_sb[2 + hg][32 * j:32 * j + 32, kb:kb + 128],
                           qk_sb[hg][32 * j:32 * j + 32, qcol:qcol + 256],
                           True, True, tp=(32 * j, 0), inc=(j == 3))
                    mark(f"sc_{u}_{c}", pe)

                def pv_r(c):
                    kc, qh = c >> 1, c & 1
                    tvt = a * 8 + kc
                    ebuf = es_sb[qh]
                    # PSUM start clears the WHOLE bank (zero-region) for the
                    # mm's partitions, so only chunk 0 may start the group;
                    # qh=1 kc=0 writes land on already-cleared bytes.
                    st, sp = (c == 0), (c == 15)
                    for j in range(4):
                        mm(u_ps[32 * j:32 * j + 32, qh * 256:qh * 256 + 256],
                           vT_sb[:, tvt * 256 + hg * 128 + 32 * j:
                                 tvt * 256 + hg * 128 + 32 * j + 32],
                           ebuf[:, j * 256:(j + 1) * 256],
                           st, sp, tp=(0, 32 * j))
                    for j in range(4):
                        mm(r_ps[32 * j:32 * j + 1, qh * 256:qh * 256 + 256],
                           ones_sb[:, 0:1], ebuf[:, j * 256:(j + 1) * 256],
                           st, sp, tp=(0, 32 * j), inc=(j == 3))
                    mark(f"pvr_{u}_{c}", pe)

                def dw_rounds(slot):
                    if "nodw" in KVAR:
                        return
                    for r in range(DW_ROUNDS):
                        if DW_SLOT[r] != slot:
                            continue
                        dx = r >> 1
                        dy0 = 0 if (r & 1) == 0 else 4
                        if r == 0:
                            conv_war(dwkey)
                            wmark(h, f"stk_half_{ct}")
                        for gi in range(4):
                            g = 4 * ct + gi
                            mm(conv_ps[dwb][32 * gi:32 * gi + 32, :],
                               dwstk_sb[g][:, r * 32:(r + 1) * 32],
                               stk3[g][:, 8 * dn + dy0:8 * dn + dy0 + 8, dx:dx + WW],
                               r == 0, r == DW_ROUNDS - 1, tp=(0, 32 * gi),
                               inc=(r == DW_ROUNDS - 1 and gi == 3))
                        if r == DW_ROUNDS - 1:
                            mark(dwkey, pe)

                # needed input tiles for this unit
                wmark(h, f"ev_qk_{2 + hg}_{2 * a + 1}")
                wmark(h, f"ev_qk_{hg}_{2 * a + 1}")
                scores(0)
                scores(1)
                for c in range(1, 17):
                    dw_rounds(c - 1)
                    wmark(h, f"exp_{u}_{c - 1}")
                    if c == 1:
                        wmark(h, f"ev_vt_{a * 8 + 7}")
                        if u > 0:
                            wmark(h, f"mul_{u - 1}")
                    pv_r(c - 1)
                    if c < 16:
                        scores(c)
                # rb broadcast (needs recip on DVE)
                wmark(h, f"recip_{u}")
                for j in range(4):
                    mm(r_ps[32 * j:32 * j + 32, :],
                       ones_sb[32 * j:32 * j + 1, 0:32],
                       rcinv_sb[32 * j:32 * j + 1, :],
                       True, True, tp=(32 * j, 32 * j), inc=(j == 3))
                mark(f"rb_{u}", pe)

                if hg == 1 and qt == 1:
                    wmark(h, f"z_{a}")
                    for nt in range(2):
                        for m in range(2):
                            key = f"pj_{a}_{nt}_{m}"
                            b = user_bank[key]
                            conv_war(key)
                            mm(conv_ps[b], wprojT_sb[0][:, m * 128:(m + 1) * 128],
                               z_sb[0][:, nt * 512:(nt + 1) * 512], True, False)
                            mm(conv_ps[b], wprojT_sb[1][:, m * 128:(m + 1) * 128],
                               z_sb[1][:, nt * 512:(nt + 1) * 512], False, True, inc=True)
                            mark(key, pe)

        # ---------------- ACT ----------------
        def act_prog(h):
            def op(fn, *args, **kw):
                if real:
                    i = fn(*args, **kw)
                    act.bump(i, real, sems)
                else:
                    act.n += 1

            # vT evac
            for t in range(32):
                key = f"vt_{t}"
                wmark(h, key)
                op(nc.scalar.activation,
                   vT_sb[:, t * 256:(t + 1) * 256],
                   conv_ps[user_bank[key]][:, 0:256], AF.Copy)
                mark(f"ev_{key}", act)
            # v -> vpad evac (after whole-buffer border memsets)
            wmark(h, "memsets_done")
            for m in range(2):
                for n in range(8):
                    key = f"v_{m}_{n}"
                    wmark(h, key)
                    op(nc.scalar.activation,
                       vpad3[m][:, 2 + 8 * n:2 + 8 * n + 8, 2:2 + WW],
                       conv_ps[user_bank[key]].rearrange("p (r w) -> p r w", r=8),
                       AF.Copy)
                    mark(f"ev_{key}", act)
                mark(f"ev_vhalf_{m}", act)
            # exp chunks
            for u in range(16):
                if "nounits" in KVAR:
                    break
                for c in range(16):
                    if c in DVE_CHUNKS:
                        continue
                    wmark(h, f"sc_{u}_{c}")
                    op(nc.scalar.activation, es_sb[c & 1], sc_ps[c & 1],
                       AF.Exp, 0.0, LN2)
                    mark(f"exp_{u}_{c}", act)

        # ---------------- DVE ----------------
        def dve_prog(h):
            def selfwait():
                if real:
                    h.wait_ge(sems["dve_s"], dve.n)

            def op(fn, *args, **kw):
                if real:
                    i = fn(*args, **kw)
                    dve.bump(i, real, sems)
                else:
                    dve.n += 1

            # border + tail memsets (2D whole-buffer: 3D strided memsets are
            # unproven on HW)
            for m in range(2):
                op(nc.vector.memset, vpad_sb[m], 0.0)
            for g in range(8):
                op(nc.vector.memset, stk_sb[g][:, NPAD - 3 * 68:NPAD], 0.0)
            op(nc.vector.memset, r_ps, 1.0)
            mark("memsets_done", dve)

            # qk evac with bias
            for m in range(4):
                for n in range(8):
                    key = f"qk_{m}_{n}"
                    wmark(h, key)
                    op(nc.vector.tensor_scalar,
                       qk_sb[m][:, n * 512:(n + 1) * 512],
                       conv_ps[user_bank[key]],
                       bias_sb[:, m:m + 1], None, ALU.add)
                    mark(f"ev_{key}", dve)

            if "nounits" in KVAR:
                for a in range(AREA):
                    for nt in range(2):
                        for m in range(2):
                            op(nc.vector.memset, out_sb[nt * 2 + m], 0.0)
                            mark(f"po_{a}_{nt}_{m}", dve)
            for u, (a, hg, qt) in enumerate(UNITS):
                if "nounits" in KVAR:
                    break
                ct, dn = hg, 2 * a + qt
                # Schraudolph exp chunks (two ops: DVE PSUM reads must stay
                # within a single 2KB bank)
                for c in DVE_CHUNKS:
                    wmark(h, f"sc_{u}_{c}")
                    op(nc.vector.tensor_scalar,
                       es_i16[c & 1][:, 0:512], sc_ps[c & 1][:, 0:512],
                       128.0, SCH_B, ALU.mult, ALU.add)
                    op(nc.vector.tensor_scalar,
                       es_i16[c & 1][:, 512:1024], sc_ps[c & 1][:, 512:1024],
                       128.0, SCH_B, ALU.mult, ALU.add)
                    mark(f"exp_{u}_{c}", dve)
                # epilogue
                wmark(h, f"pvr_{u}_15")
                op(nc.vector.tensor_copy, ucp_sb, u_ps)
                if real:
                    lp = nc.allow_low_precision("softmax denom to bf16")
                    lp.__enter__()
                op(nc.vector.reciprocal, rcinv_sb, r_ps)
                if real:
                    lp.__exit__(None, None, None)
                mark(f"recip_{u}", dve)
                wmark(h, f"rb_{u}")
                selfwait()
                op(nc.vector.tensor_mul,
                   oT_sb[hg][:, qt * 512:qt * 512 + 512], ucp_sb, r_ps)
                mark(f"mul_{u}", dve)
                # dw evac
                if "nodw" in KVAR:
                    op(nc.vector.memset, pp_sb[ct][:, qt * 512:qt * 512 + 512], 0.0)
                else:
                    wmark(h, f"dw_{u}")
                    op(nc.vector.tensor_copy,
                       pp_sb[ct][:, qt * 512:qt * 512 + 512],
                       conv_ps[user_bank[f"dw_{u}"]])
                mark(f"ev_dw_{u}", dve)

                if hg == 1 and qt == 1:
                    if a > 0:
                        wmark(h, f"pj_{a - 1}_1_1")
                    selfwait()
                    for cti in range(2):
                        op(nc.vector.tensor_add,
                           z_sb[cti], oT_sb[cti], pp_sb[cti])
                    mark(f"z_{a}", dve)
                    for nt in range(2):
                        for m in range(2):
                            key = f"pj_{a}_{nt}_{m}"
                            wmark(h, key)
                            if a > 0:
                                wraw(h, f"dma_o{nt * 2 + m}", 16 * a)
                            op(nc.vector.tensor_scalar,
                               out_sb[nt * 2 + m], conv_ps[user_bank[key]],
                               bias_sb[:, 4 + m:5 + m], None, ALU.add)
                            mark(f"ev_{key}", dve)
                            mark(f"po_{a}_{nt}_{m}", dve)

        if real:
            with nc.Block() as block, \
                 nc.semaphore("dma_i") as s_dma_i, \
                 nc.semaphore("dma_k0") as s_dma_k0, \
                 nc.semaphore("dma_k1") as s_dma_k1, \
                 nc.semaphore("dma_o0") as s_dma_o0, \
                 nc.semaphore("dma_o1") as s_dma_o1, \
                 nc.semaphore("dma_o2") as s_dma_o2, \
                 nc.semaphore("dma_o3") as s_dma_o3, \
                 nc.semaphore("pe_s") as s_pe, \
                 nc.semaphore("act_s") as s_act, \
                 nc.semaphore("dve_s") as s_dve:
                sems.update({"dma_i": s_dma_i,
                             "dma_k0": s_dma_k0, "dma_k1": s_dma_k1,
                             "dma_o0": s_dma_o0, "dma_o1": s_dma_o1,
                             "dma_o2": s_dma_o2, "dma_o3": s_dma_o3,
                             "pe_s": s_pe, "act_s": s_act, "dve_s": s_dve})

                @block.sync
                def _(sync):
                    sp_prog(sync)

                @block.tensor
                def _(tensor):
                    pe_prog(tensor)

                @block.scalar
                def _(scalar):
                    act_prog(scalar)

                @block.vector
                def _(vector):
                    dve_prog(vector)
        else:
            class H:
                def wait_ge(self, *a, **k):
                    pass

                def dma_start(self, *a, **k):
                    class R:
                        def then_inc(self, *a, **k):
                            return self
                    return R()
            hh = H()
            sp_prog(hh)
            pe_prog(hh)
            act_prog(hh)
            dve_prog(hh)

    def make_engines():
        return {"pe": Eng("pe", "pe_s"), "act": Eng("act", "act_s"),
                "dve": Eng("dve", "dve_s"), "sp": Eng("sp", "dma_i")}

    sems = {}
    program(False, make_engines(), sems)    # dry: fill marks
    program(True, make_engines(), sems)     # real emission
    return nc


@functools.lru_cache(maxsize=1)
def _get_nc():
    return _build_nc()


def _prep_host(inputs):
    x = np.asarray(inputs["x"], np.float32)            # [8, 256, 64, 64]
    w_qk = np.asarray(inputs["w_qk"], np.float32)      # [512, 256]
    s_qk = np.asarray(inputs["s_qk"], np.float32)
    b_qk = np.asarray(inputs["b_qk"], np.float32)
    w_v = np.asarray(inputs["w_v"], np.float32)
    s_v = np.asarray(inputs["s_v"], np.float32)
    b_v = np.asarray(inputs["b_v"], np.float32)
    w_pe = np.asarray(inputs["w_pe"], np.float32)      # [256, 1, 5, 5]
    s_pe = np.asarray(inputs["s_pe"], np.float32)
    b_pe = np.asarray(inputs["b_pe"], np.float32)
    w_proj = np.asarray(inputs["w_proj"], np.float32)
    s_proj = np.asarray(inputs["s_proj"], np.float32)
    b_proj = np.asarray(inputs["b_proj"], np.float32)

    # fold BN scales into weights; fold 1/sqrt(d)*log2(e) into q weights+bias
    # (scores come out as t = log2(e) * s; ACT exp uses scale=ln2, DVE uses
    # the Schraudolph bit trick on t directly)
    w_qk_eff = w_qk * s_qk[:, None]
    b_qk_eff = b_qk.copy()      # reference: y = (Wx)*s + b, bias not scaled
    w_qk_eff[:C] *= SCALE * LOG2E
    b_qk_eff[:C] *= SCALE * LOG2E

    w_v_eff = w_v * s_v[:, None]
    w_proj_eff = w_proj * s_proj[:, None]

    wpe = w_pe.reshape(C, 25)                          # [c, tap] (dy*5+dx)
    wpe_eff = wpe * s_pe[:, None]

    # constants folded through attention/depthwise into proj bias
    kappa = b_v + s_pe * b_v * wpe.sum(1) + b_pe       # [256]
    b_proj_eff = b_proj + w_proj_eff @ kappa

    # dw stacked-tap weights: per 32-chan group g, round r, block t:
    #  r even (dx=r//2): tap (dy=t, dx); r odd: tap (4, dx) on block 0 only
    dwstk = np.zeros((8, 128, DW_ROUNDS * 32), np.float32)
    for g in range(8):
        for r in range(DW_ROUNDS):
            dx = r >> 1
            if (r & 1) == 0:
                for t in range(4):
                    for i in range(32):
                        dwstk[g, 32 * t + i, r * 32 + i] = wpe_eff[g * 32 + i, t * 5 + dx]
            else:
                for i in range(32):
                    dwstk[g, i, r * 32 + i] = wpe_eff[g * 32 + i, 20 + dx]

    bias_tab = np.zeros((128, 8), np.float32)
    for m in range(4):
        bias_tab[:, m] = b_qk_eff[m * 128:(m + 1) * 128]
    for m in range(2):
        bias_tab[:, 4 + m] = b_proj_eff[m * 128:(m + 1) * 128]

    common = {
        "wqkT": np.ascontiguousarray(w_qk_eff.T).astype(BF16NP),
        "wvT": np.ascontiguousarray(w_v_eff.T).astype(BF16NP),
        "wprojT": np.ascontiguousarray(w_proj_eff.T).astype(BF16NP),
        "dwstk": dwstk.astype(BF16NP),
        "ones": np.ones((128, 512), BF16NP),
        "bias": bias_tab,
    }
    in_maps = []
    for i in range(8):
        m = dict(common)
        m["x"] = np.ascontiguousarray(x[i].reshape(C, NTOK)).astype(BF16NP)
        in_maps.append(m)
    return in_maps


def kernel(**inputs):
    nc = _get_nc()
    in_maps = _prep_host(inputs)
    res = run_bass_kernel_spmd(nc, in_maps, core_ids=list(range(8)))
    outs = [res.results[i]["out"].reshape(C, HH, WW) for i in range(8)]
    return np.stack(outs, 0).astype(np.float32)


if __name__ == "__main__":
    rng = np.random.default_rng(0)
    fake = {
        "x": rng.standard_normal((8, C, HH, WW), np.float32),
        "w_qk": rng.standard_normal((2 * C, C), np.float32) * 0.05,
        "s_qk": np.ones(2 * C, np.float32),
        "b_qk": rng.standard_normal(2 * C).astype(np.float32) * 0.01,
        "w_v": rng.standard_normal((C, C), np.float32) * 0.05,
        "s_v": np.ones(C, np.float32),
        "b_v": rng.standard_normal(C).astype(np.float32) * 0.01,
        "w_pe": rng.standard_normal((C, 1, 5, 5), np.float32) * 0.05,
        "s_pe": np.ones(C, np.float32),
        "b_pe": rng.standard_normal(C).astype(np.float32) * 0.01,
        "w_proj": rng.standard_normal((C, C), np.float32) * 0.05,
        "s_proj": np.ones(C, np.float32),
        "b_proj": rng.standard_normal(C).astype(np.float32) * 0.01,
    }
    out = kernel(**fake)
    print("out", out.shape, out.dtype, float(np.abs(out).mean()))


# revision 19
# speedup vs baseline: 1.3410x; 1.0815x over previous
"""Trainium2 Bass kernel for area-attention (YOLOv12 A2-style) module.

Raw-bass SPMD: 8 cores, 1 image each, no collectives. Engine split:
SP=DMA, PE=all matmuls (convs, dw-diag conv, attention with PE-array
row/col packing), ACT=softmax exp, DVE=evac/normalize/memset.
Software-pipelined per (area, head-group, q-tile) unit: exp(kc)
overlaps PV(kc-1)+scores(kc)+depthwise filler matmuls.
"""

import sys, os, functools

sys.path.insert(0, "/opt/trn_rl_repo")

import numpy as np
import ml_dtypes

import concourse.bass as bass
import concourse.mybir as mybir
from concourse.bass_utils import run_bass_kernel_spmd

BF16NP = ml_dtypes.bfloat16
F32 = mybir.dt.float32
BF = mybir.dt.bfloat16
I16 = mybir.dt.int16
ALU = mybir.AluOpType
SCH_K = 128.0 * float(np.log2(np.e))   # fold log2e into the DVE affine
SCH_B = 16248.636                      # mean-centered Schraudolph bias

C = 256
HH = 64
WW = 64
NTOK = HH * WW          # 4096
AREA = 4
NA = NTOK // AREA       # 1024
D = 32                  # head dim
SCALE = float(D) ** -0.5
PADW = WW + 4           # 68
PADH = HH + 4           # 68
NPAD = PADW * PADH      # 4624
AF = mybir.ActivationFunctionType

UNITS = [(a, hg, qt) for a in range(AREA) for hg in range(2) for qt in range(2)]
# dw output tile needed by proj of area a: (ct, 2a+qt)
DW_TILES = [(hg, 2 * a + qt) for a in range(AREA) for hg in range(2) for qt in range(2)]
N_IN_DMA = 12


def _build_nc():
    import contextlib
    nc = bass.Bass()

    x_d = nc.declare_dram_parameter("x", [C, NTOK], BF, isOutput=False)
    wqkT_d = nc.declare_dram_parameter("wqkT", [C, 2 * C], BF, isOutput=False)
    wvT_d = nc.declare_dram_parameter("wvT", [C, C], BF, isOutput=False)
    wprojT_d = nc.declare_dram_parameter("wprojT", [C, C], BF, isOutput=False)
    dwdiag_d = nc.declare_dram_parameter("dwdiag", [2, 128, 25 * 128], BF, isOutput=False)
    ones_d = nc.declare_dram_parameter("ones", [128, 512], BF, isOutput=False)
    brows_d = nc.declare_dram_parameter("brows", [1, 1024], BF, isOutput=False)
    out_d = nc.declare_dram_parameter("out", [C, NTOK], F32, isOutput=True)

    ctx = contextlib.ExitStack()
    _names = [0]

    def sb(shape, dtype, nm=None):
        _names[0] += 1
        return ctx.enter_context(nc.sbuf_tensor(f"sb{_names[0]}", shape, dtype))[:, :]

    def ps(shape):
        _names[0] += 1
        return ctx.enter_context(nc.psum_tensor(f"ps{_names[0]}", shape, F32))[:, :]

    x_sb = [sb([128, NTOK], BF) for _ in range(2)]
    qk_sb = [sb([128, NTOK], BF) for _ in range(4)]
    vT_sb = sb([128, 32 * 256], BF)
    vpad_sb = [sb([128, NPAD], BF) for _ in range(2)]
    pp_sb = [sb([128, NTOK], BF) for _ in range(2)]
    oT_sb = [sb([128, NTOK], BF) for _ in range(2)]
    z_sb = [sb([128, NTOK], BF) for _ in range(2)]
    wqkT_sb = [sb([128, 2 * C], BF) for _ in range(2)]
    wvT_sb = [sb([128, C], BF) for _ in range(2)]
    wprojT_sb = [sb([128, C], BF) for _ in range(2)]
    dw_sb = [sb([128, 25 * 128], BF) for _ in range(2)]
    ones_sb = sb([128, 512], BF)
    brows_sb = sb([1, 1024], BF)
    rcinv_sb = sb([128, 512], BF)
    es_sb = [sb([128, 1536], BF) for _ in range(2)]  # heads 0-2 (ACT exp)
    esd_sb = [sb([128, 512], BF) for _ in range(2)]  # head 3 (DVE Schraudolph)
    ucp_sb = sb([128, 512], F32)
    out_sb = [sb([128, 512], F32) for _ in range(16)]

    sc_ps = ps([128, 2048])   # 4 banks
    u_ps = ps([128, 512])
    r_ps = ps([128, 512])
    rb_ps = r_ps  # shared bank: recip reads r before bcast overwrites (sem-ordered)
    conv_ps = [ps([128, 512]) for _ in range(2)]

    vpad3 = [vp.rearrange("p (h w) -> p h w", h=PADH) for vp in vpad_sb]

    marks = {}  # name -> counter value (producer engine count at completion)

    # conv-bank users in PE order: list of keys; user i uses bank i%2 and
    # must WAR-wait on evac of user i-2 (marks['ev_'+key]).
    conv_users = []
    for m in range(4):
        for n in range(8):
            conv_users.append(f"qk_{m}_{n}")
    for t in range(32):
        conv_users.append(f"vt_{t}")
    for m in range(2):
        for n in range(8):
            conv_users.append(f"v_{m}_{n}")
    # dw tile u interleaved within unit u; proj tiles after each area
    seq = []
    for u, (a, hg, qt) in enumerate(UNITS):
        seq.append(f"dw_{u}")
        if hg == 1 and qt == 1:
            for nt in range(2):
                for m in range(2):
                    seq.append(f"pj_{2 * a + nt}_{m}")
    conv_users.extend(seq)
    user_bank = {k: i % 2 for i, k in enumerate(conv_users)}
    user_prev = {k: (conv_users[i - 2] if i >= 2 else None)
                 for i, k in enumerate(conv_users)}

    class Eng:
        """Counts instructions; in real mode also emits via `fns`."""

        def __init__(self, name, sem_name):
            self.name = name
            self.sem_name = sem_name
            self.n = 0

        def bump(self, binst, real, sems):
            self.n += 1
            if real:
                binst.then_inc(sems[self.sem_name], 1)

    def program(real, engines, sems):
        pe, act, dve, sp = engines["pe"], engines["act"], engines["dve"], engines["sp"]

        def w(eng_handle, sem_name, val):
            # standalone wait_ge; no count
            if real and val is not None and val > 0:
                eng_handle.wait_ge(sems[sem_name], val)

        def mark(name, eng):
            if not real:
                marks[name] = eng.n

        def get(name):
            return marks.get(name, 0)

        # ---------------- SP ----------------
        def sp_prog(h):
            n_dma = 0
            if real:
                for i in range(2):
                    h.dma_start(out=wqkT_sb[i], in_=wqkT_d[i * 128:(i + 1) * 128, :]).then_inc(sems["dma_i"], 16)
                    h.dma_start(out=wvT_sb[i], in_=wvT_d[i * 128:(i + 1) * 128, :]).then_inc(sems["dma_i"], 16)
                    h.dma_start(out=wprojT_sb[i], in_=wprojT_d[i * 128:(i + 1) * 128, :]).then_inc(sems["dma_i"], 16)
                    h.dma_start(out=dw_sb[i], in_=dwdiag_d[i, :, :]).then_inc(sems["dma_i"], 16)
                h.dma_start(out=ones_sb, in_=ones_d[:, :]).then_inc(sems["dma_i"], 16)
                h.dma_start(out=brows_sb, in_=brows_d[:, :]).then_inc(sems["dma_i"], 16)
                for i in range(2):
                    h.dma_start(out=x_sb[i], in_=x_d[i * 128:(i + 1) * 128, :]).then_inc(sems["dma_i"], 16)
            n_dma = 12
            assert n_dma == N_IN_DMA
            # out DMAs
            for k in range(8):            # token tile n
                for m in range(2):
                    idx = k * 2 + m
                    if real:
                        h.wait_ge(sems["dve_s"], marks[f"po_{k}_{m}"])
                        h.dma_start(out=out_d[m * 128:(m + 1) * 128, k * 512:(k + 1) * 512],
                                    in_=out_sb[idx]).then_inc(sems["dma_o"], 16)
            if real:
                h.wait_ge(sems["dma_o"], 16 * 16)

        # ---------------- PE ----------------
        def pe_prog(h):
            def mm(out, lhsT, rhs, start, stop, tp=None):
                if real:
                    i = nc.tensor.matmul(out, lhsT, rhs, start=start, stop=stop,
                                         tile_position=tp, skip_group_check=True)
                    pe.bump(i, real, sems)
                else:
                    pe.n += 1

            def conv_war(key):
                prev = user_prev[key]
                if prev is not None:
                    w(h, "dve_s", get(f"ev_{prev}"))

            w(h, "dma_i", N_IN_DMA * 16)
            # qk GEMM
            for m in range(4):
                for n in range(8):
                    key = f"qk_{m}_{n}"
                    b = user_bank[key]
                    conv_war(key)
                    mm(conv_ps[b], wqkT_sb[0][:, m * 128:(m + 1) * 128],
                       x_sb[0][:, n * 512:(n + 1) * 512], True, False)
                    mm(conv_ps[b], wqkT_sb[1][:, m * 128:(m + 1) * 128],
                       x_sb[1][:, n * 512:(n + 1) * 512], False, False)
                    mm(conv_ps[b], brows_sb[0:1, m * 128:(m + 1) * 128],
                       ones_sb[0:1, 0:512], False, True)
                    mark(key, pe)
            # vT GEMM
            for t in range(32):
                key = f"vt_{t}"
                b = user_bank[key]
                conv_war(key)
                mm(conv_ps[b][:, 0:256], x_sb[0][:, t * 128:(t + 1) * 128], wvT_sb[0], True, False)
                mm(conv_ps[b][:, 0:256], x_sb[1][:, t * 128:(t + 1) * 128], wvT_sb[1], False, True)
                mark(key, pe)
            # v GEMM
            for m in range(2):
                for n in range(8):
                    key = f"v_{m}_{n}"
                    b = user_bank[key]
                    conv_war(key)
                    mm(conv_ps[b], wvT_sb[0][:, m * 128:(m + 1) * 128],
                       x_sb[0][:, n * 512:(n + 1) * 512], True, False)
                    mm(conv_ps[b], wvT_sb[1][:, m * 128:(m + 1) * 128],
                       x_sb[1][:, n * 512:(n + 1) * 512], False, True)
                    mark(key, pe)
            # wait all startup evacs (qk ready for scores, vT for PV, vpad for dw)
            w(h, "dve_s", get("startup_evac"))

            for u, (a, hg, qt) in enumerate(UNITS):
                qb = a * NA + qt * 512

                def scores(kc):
                    kb = a * NA + kc * 128
                    for j in range(4):
                        mm(sc_ps[:, j * 512:(j + 1) * 512],
                           qk_sb[2 + hg][32 * j:32 * j + 32, kb:kb + 128],
                           qk_sb[hg][32 * j:32 * j + 32, qb:qb + 512],
                           True, True, tp=(32 * j, 0))

                def pv(kc):
                    tvt = a * 8 + kc
                    for j in range(4):
                        if j < 3:
                            esj = es_sb[kc % 2][:, j * 512:(j + 1) * 512]
                        else:
                            esj = esd_sb[kc % 2][:, :]
                        mm(u_ps[32 * j:32 * j + 32, :],
                           vT_sb[:, tvt * 256 + hg * 128 + 32 * j:
                                 tvt * 256 + hg * 128 + 32 * j + 32],
                           esj, kc == 0, kc == 7, tp=(0, 32 * j))
                    for j in range(4):
                        if j < 3:
                            esj = es_sb[kc % 2][:, j * 512:(j + 1) * 512]
                        else:
                            esj = esd_sb[kc % 2][:, :]
                        mm(r_ps[32 * j:32 * j + 1, :], ones_sb[:, 0:1], esj,
                           kc == 0, kc == 7, tp=(0, 32 * j))

                # dw filler chunks for dw tile u
                ct, dn = DW_TILES[u]
                dwkey = f"dw_{u}"
                dwb = user_bank[dwkey]

                def dw_chunk(ci):
                    taps = range((25 * ci) // 8, (25 * (ci + 1)) // 8)
                    for tap in taps:
                        if tap == 0:
                            conv_war(dwkey)
                        dy, dx = divmod(tap, 5)
                        mm(conv_ps[dwb], dw_sb[ct][:, tap * 128:(tap + 1) * 128],
                           vpad3[ct][:, 8 * dn + dy:8 * dn + dy + 8, dx:dx + WW],
                           tap == 0, tap == 24)
                    if taps and max(taps) == 24:
                        mark(dwkey, pe)

                # unit prologue: WAR on U/r/rb banks vs previous unit's DVE reads
                if u > 0:
                    w(h, "dve_s", get(f"unit_dve_{u - 1}"))
                scores(0)
                mark(f"grp_{u}_0", pe)
                for kc in range(1, 9):
                    dw_chunk(kc - 1)
                    w(h, "act_s", get(f"exp_{u}_{kc - 1}"))
                    w(h, "dve_s", get(f"expd_{u}_{kc - 1}"))
                    pv(kc - 1)
                    if kc < 8:
                        scores(kc)
                        mark(f"grp_{u}_{kc}", pe)
                mark(f"unitpv_{u}", pe)
                # rb broadcast (needs recip on DVE)
                w(h, "dve_s", get(f"recip_{u}"))
                for j in range(4):
                    mm(rb_ps[32 * j:32 * j + 32, :],
                       ones_sb[32 * j:32 * j + 1, 0:32],
                       rcinv_sb[32 * j:32 * j + 1, :],
                       True, True, tp=(32 * j, 32 * j))
                mark(f"rb_{u}", pe)

                if hg == 1 and qt == 1:
                    # proj for area a
                    w(h, "dve_s", get(f"z_{a}"))
                    for nt in range(2):
                        k = 2 * a + nt
                        for m in range(2):
                            key = f"pj_{k}_{m}"
                            b = user_bank[key]
                            conv_war(key)
                            mm(conv_ps[b], wprojT_sb[0][:, m * 128:(m + 1) * 128],
                               z_sb[0][:, k * 512:(k + 1) * 512], True, False)
                            mm(conv_ps[b], wprojT_sb[1][:, m * 128:(m + 1) * 128],
                               z_sb[1][:, k * 512:(k + 1) * 512], False, False)
                            mm(conv_ps[b], brows_sb[0:1, 512 + m * 128:512 + (m + 1) * 128],
                               ones_sb[0:1, 0:512], False, True)
                            mark(key, pe)

        # ---------------- ACT ----------------
        def act_prog(h):
            def ex(out, in_):
                if real:
                    i = nc.scalar.activation(out, in_, AF.Exp)
                    act.bump(i, real, sems)
                else:
                    act.n += 1

            for u, (a, hg, qt) in enumerate(UNITS):
                for kc in range(8):
                    w(h, "pe_s", get(f"grp_{u}_{kc}"))
                    ex(es_sb[kc % 2], sc_ps[:, 0:1536])
                    mark(f"exp_{u}_{kc}", act)

        # ---------------- DVE ----------------
        def dve_prog(h):
            def selfwait():
                if real:
                    h.wait_ge(sems["dve_s"], dve.n)

            def op(fn, *args, **kw):
                if real:
                    i = fn(*args, **kw)
                    dve.bump(i, real, sems)
                else:
                    dve.n += 1

            op(nc.vector.memset, vpad_sb[0], 0.0)
            op(nc.vector.memset, vpad_sb[1], 0.0)
            op(nc.vector.memset, r_ps, 1.0)

            for m in range(4):
                for n in range(8):
                    key = f"qk_{m}_{n}"
                    w(h, "pe_s", get(key))
                    op(nc.vector.tensor_copy,
                       qk_sb[m][:, n * 512:(n + 1) * 512], conv_ps[user_bank[key]])
                    mark(f"ev_{key}", dve)
            for t in range(32):
                key = f"vt_{t}"
                w(h, "pe_s", get(key))
                op(nc.vector.tensor_copy,
                   vT_sb[:, t * 256:(t + 1) * 256], conv_ps[user_bank[key]][:, 0:256])
                mark(f"ev_{key}", dve)
            selfwait()
            for m in range(2):
                for n in range(8):
                    key = f"v_{m}_{n}"
                    w(h, "pe_s", get(key))
                    op(nc.vector.tensor_copy,
                       vpad3[m][:, 2 + 8 * n:2 + 8 * n + 8, 2:2 + WW],
                       conv_ps[user_bank[key]].rearrange("p (r w) -> p r w", r=8))
                    mark(f"ev_{key}", dve)
            mark("startup_evac", dve)

            for u, (a, hg, qt) in enumerate(UNITS):
                qb = a * NA + qt * 512
                # head-3 exp via Schraudolph bit trick (single-bank PSUM read,
                # whole-tensor bitcast)
                for kc in range(8):
                    w(h, "pe_s", get(f"grp_{u}_{kc}"))
                    op(nc.vector.tensor_scalar,
                       esd_sb[kc % 2].bitcast(I16), sc_ps[:, 1536:2048],
                       SCH_K, SCH_B, ALU.mult, ALU.add)
                    mark(f"expd_{u}_{kc}", dve)
                w(h, "pe_s", get(f"unitpv_{u}"))
                if real:
                    lp = nc.allow_low_precision("softmax denom to bf16")
                    lp.__enter__()
                selfwait()
                op(nc.vector.reciprocal, rcinv_sb, r_ps)
                if real:
                    lp.__exit__(None, None, None)
                mark(f"recip_{u}", dve)
                op(nc.vector.tensor_copy, ucp_sb, u_ps)
                w(h, "pe_s", get(f"rb_{u}"))
                selfwait()
                op(nc.vector.tensor_mul,
                   oT_sb[hg][:, qb:qb + 512], ucp_sb, rb_ps)
                mark(f"unit_dve_{u}", dve)

                # dw evac for tile u
                ct, dn = DW_TILES[u]
                dwkey = f"dw_{u}"
                w(h, "pe_s", get(dwkey))
                op(nc.vector.tensor_copy,
                   pp_sb[ct][:, dn * 512:(dn + 1) * 512], conv_ps[user_bank[dwkey]])
                mark(f"ev_{dwkey}", dve)

                if hg == 1 and qt == 1:
                    # z = o + pp for area a
                    selfwait()
                    for cti in range(2):
                        op(nc.vector.tensor_add,
                           z_sb[cti][:, a * NA:(a + 1) * NA],
                           oT_sb[cti][:, a * NA:(a + 1) * NA],
                           pp_sb[cti][:, a * NA:(a + 1) * NA])
                    mark(f"z_{a}", dve)
                    # proj evacs
                    for nt in range(2):
                        k = 2 * a + nt
                        for m in range(2):
                            key = f"pj_{k}_{m}"
                            idx = k * 2 + m
                            w(h, "pe_s", get(key))
                            op(nc.vector.tensor_copy, out_sb[idx],
                               conv_ps[user_bank[key]])
                            mark(f"ev_{key}", dve)
                            mark(f"po_{k}_{m}", dve)

        if real:
            with nc.Block() as block, \
                 nc.semaphore("dma_i") as s_dma_i, \
                 nc.semaphore("dma_o") as s_dma_o, \
                 nc.semaphore("pe_s") as s_pe, \
                 nc.semaphore("act_s") as s_act, \
                 nc.semaphore("dve_s") as s_dve:
                sems.update({"dma_i": s_dma_i, "dma_o": s_dma_o,
                             "pe_s": s_pe, "act_s": s_act, "dve_s": s_dve})

                @block.sync
                def _(sync):
                    sp_prog(sync)

                @block.tensor
                def _(tensor):
                    pe_prog(tensor)

                @block.scalar
                def _(scalar):
                    act_prog(scalar)

                @block.vector
                def _(vector):
                    dve_prog(vector)
        else:
            class H:  # dry handle
                def wait_ge(self, *a, **k):
                    pass

                def dma_start(self, *a, **k):
                    class R:
                        def then_inc(self, *a, **k):
                            return self
                    return R()
            hh = H()
            sp_prog(hh)
            pe_prog(hh)
            act_prog(hh)
            dve_prog(hh)

    engines = {"pe": Eng("pe", "pe_s"), "act": Eng("act", "act_s"),
               "dve": Eng("dve", "dve_s"), "sp": Eng("sp", "dma_i")}
    sems = {}
    program(False, engines, sems)          # dry: fill marks
    engines = {"pe": Eng("pe", "pe_s"), "act": Eng("act", "act_s"),
               "dve": Eng("dve", "dve_s"), "sp": Eng("sp", "dma_i")}
    program(True, engines, sems)           # real emission
    return nc


@functools.lru_cache(maxsize=1)
def _get_nc():
    return _build_nc()


def _prep_host(inputs):
    x = np.asarray(inputs["x"], np.float32)            # [8, 256, 64, 64]
    w_qk = np.asarray(inputs["w_qk"], np.float32)      # [512, 256]
    s_qk = np.asarray(inputs["s_qk"], np.float32)
    b_qk = np.asarray(inputs["b_qk"], np.float32)
    w_v = np.asarray(inputs["w_v"], np.float32)
    s_v = np.asarray(inputs["s_v"], np.float32)
    b_v = np.asarray(inputs["b_v"], np.float32)
    w_pe = np.asarray(inputs["w_pe"], np.float32)      # [256, 1, 5, 5]
    s_pe = np.asarray(inputs["s_pe"], np.float32)
    b_pe = np.asarray(inputs["b_pe"], np.float32)
    w_proj = np.asarray(inputs["w_proj"], np.float32)
    s_proj = np.asarray(inputs["s_proj"], np.float32)
    b_proj = np.asarray(inputs["b_proj"], np.float32)

    # fold BN scales into weights; fold 1/sqrt(d) into q weights+bias
    w_qk_eff = w_qk * s_qk[:, None]
    b_qk_eff = b_qk * s_qk  # BN affine: y = s*(Wx) + b ... b is already the bias
    # NB: reference _conv1x1 computes  y = (Wx)*s + b, so bias is NOT scaled by s.
    b_qk_eff = b_qk.copy()
    w_qk_eff[:C] *= SCALE
    b_qk_eff[:C] *= SCALE

    w_v_eff = w_v * s_v[:, None]
    w_proj_eff = w_proj * s_proj[:, None]

    wpe = w_pe.reshape(C, 25)                          # [c, tap]
    wpe_eff = wpe * s_pe[:, None]

    # constants folded through attention/depthwise into proj bias:
    # o gets +b_v exactly (softmax rows sum to 1);
    # pp = s_pe*dw(v_nb) + s_pe*b_v*sum_taps(w_pe) + b_pe
    kappa = b_v + s_pe * b_v * wpe.sum(1) + b_pe       # [256]
    b_proj_eff = b_proj + w_proj_eff @ kappa

    dwdiag = np.zeros((2, 128, 25 * 128), np.float32)
    for ct in range(2):
        for tap in range(25):
            idx = np.arange(128)
            dwdiag[ct, idx, tap * 128 + idx] = wpe_eff[ct * 128 + idx, tap]

    common = {
        "wqkT": np.ascontiguousarray(w_qk_eff.T).astype(BF16NP),

        "wvT": np.ascontiguousarray(w_v_eff.T).astype(BF16NP),
        "wprojT": np.ascontiguousarray(w_proj_eff.T).astype(BF16NP),
        "brows": np.concatenate([b_qk_eff, b_proj_eff, np.zeros(256, np.float32)]
                                ).reshape(1, 1024).astype(BF16NP),
        "dwdiag": dwdiag.astype(BF16NP),
        "ones": np.ones((128, 512), BF16NP),
    }
    in_maps = []
    for i in range(8):
        m = dict(common)
        m["x"] = np.ascontiguousarray(x[i].reshape(C, NTOK)).astype(BF16NP)
        in_maps.append(m)
    return in_maps


def kernel(**inputs):
    nc = _get_nc()
    in_maps = _prep_host(inputs)
    res = run_bass_kernel_spmd(nc, in_maps, core_ids=list(range(8)))
    outs = [res.results[i]["out"].reshape(C, HH, WW) for i in range(8)]
    return np.stack(outs, 0).astype(np.float32)


if __name__ == "__main__":
    rng = np.random.default_rng(0)
    fake = {
        "x": rng.standard_normal((8, C, HH, WW), np.float32),
        "w_qk": rng.standard_normal((2 * C, C), np.float32) * 0.05,
        "s_qk": np.ones(2 * C, np.float32),
        "b_qk": rng.standard_normal(2 * C).astype(np.float32) * 0.01,
        "w_v": rng.standard_normal((C, C), np.float32) * 0.05,
        "s_v": np.ones(C, np.float32),
        "b_v": rng.standard_normal(C).astype(np.float32) * 0.01,
        "w_pe": rng.standard_normal((C, 1, 5, 5), np.float32) * 0.05,
        "s_pe": np.ones(C, np.float32),
        "b_pe": rng.standard_normal(C).astype(np.float32) * 0.01,
        "w_proj": rng.standard_normal((C, C), np.float32) * 0.05,
        "s_proj": np.ones(C, np.float32),
        "b_proj": rng.standard_normal(C).astype(np.float32) * 0.01,
    }
    out = kernel(**fake)
    print("out", out.shape, out.dtype, float(np.abs(out).mean()))



# revision 20
# speedup vs baseline: 1.4206x; 1.0593x over previous
"""Trainium2 Bass kernel for area-attention (YOLOv12 A2-style) module.

Raw-bass SPMD: 8 cores, 1 image each, no collectives. Engine split:
SP=DMA, PE=all matmuls (convs, dw-diag conv, attention with PE-array
row/col packing), ACT=softmax exp, DVE=evac/normalize/memset.
Software-pipelined per (area, head-group, q-tile) unit: exp(kc)
overlaps PV(kc-1)+scores(kc)+depthwise filler matmuls.
"""

import sys, os, functools

sys.path.insert(0, "/opt/trn_rl_repo")

import numpy as np
import ml_dtypes

import concourse.bass as bass
import concourse.mybir as mybir
from concourse.bass_utils import run_bass_kernel_spmd

BF16NP = ml_dtypes.bfloat16
F32 = mybir.dt.float32
BF = mybir.dt.bfloat16
I16 = mybir.dt.int16
ALU = mybir.AluOpType
SCH_K = 128.0 * float(np.log2(np.e))   # fold log2e into the DVE affine
SCH_B = 16248.636                      # mean-centered Schraudolph bias

C = 256
HH = 64
WW = 64
NTOK = HH * WW          # 4096
AREA = 4
NA = NTOK // AREA       # 1024
D = 32                  # head dim
SCALE = float(D) ** -0.5
PADW = WW + 4           # 68
PADH = HH + 4           # 68
NPAD = PADW * PADH      # 4624
AF = mybir.ActivationFunctionType

UNITS = [(a, hg, qt) for a in range(AREA) for hg in range(2) for qt in range(2)]
# dw output tile needed by proj of area a: (ct, 2a+qt)
DW_TILES = [(hg, 2 * a + qt) for a in range(AREA) for hg in range(2) for qt in range(2)]
N_IN_DMA = 12


def _build_nc():
    import contextlib
    nc = bass.Bass()

    x_d = nc.declare_dram_parameter("x", [C, NTOK], BF, isOutput=False)
    wqkT_d = nc.declare_dram_parameter("wqkT", [C, 2 * C], BF, isOutput=False)
    wvT_d = nc.declare_dram_parameter("wvT", [C, C], BF, isOutput=False)
    wprojT_d = nc.declare_dram_parameter("wprojT", [C, C], BF, isOutput=False)
    dwdiag_d = nc.declare_dram_parameter("dwdiag", [2, 128, 25 * 128], BF, isOutput=False)
    ones_d = nc.declare_dram_parameter("ones", [128, 512], BF, isOutput=False)
    brows_d = nc.declare_dram_parameter("brows", [1, 1024], BF, isOutput=False)
    out_d = nc.declare_dram_parameter("out", [C, NTOK], F32, isOutput=True)

    ctx = contextlib.ExitStack()
    _names = [0]

    def sb(shape, dtype, nm=None):
        _names[0] += 1
        return ctx.enter_context(nc.sbuf_tensor(f"sb{_names[0]}", shape, dtype))[:, :]

    def ps(shape):
        _names[0] += 1
        return ctx.enter_context(nc.psum_tensor(f"ps{_names[0]}", shape, F32))[:, :]

    x_sb = [sb([128, NTOK], BF) for _ in range(2)]
    qk_sb = [sb([128, NTOK], BF) for _ in range(4)]
    vT_sb = sb([128, 32 * 256], BF)
    vpad_sb = [sb([128, NPAD], BF) for _ in range(2)]
    pp_sb = [sb([128, NTOK], BF) for _ in range(2)]
    oT_sb = [sb([128, NTOK], BF) for _ in range(2)]
    z_sb = [sb([128, NTOK], BF) for _ in range(2)]
    wqkT_sb = [sb([128, 2 * C], BF) for _ in range(2)]
    wvT_sb = [sb([128, C], BF) for _ in range(2)]
    wprojT_sb = [sb([128, C], BF) for _ in range(2)]
    dw_sb = [sb([128, 25 * 128], BF) for _ in range(2)]
    ones_sb = sb([128, 512], BF)
    brows_sb = sb([1, 1024], BF)
    rcinv_sb = sb([128, 512], BF)
    es_sb = [sb([128, 1536], BF) for _ in range(2)]   # heads 0-2 (ACT exp)
    esd_sb = [sb([128, 512], BF) for _ in range(2)]   # head 3 (DVE Schraudolph)
    ucp_sb = sb([128, 512], F32)
    out_sb = [sb([128, 512], F32) for _ in range(16)]

    sc_ps = ps([128, 2048])   # 4 banks
    u_ps = ps([128, 512])
    r_ps = ps([128, 512])
    rb_ps = r_ps  # shared bank: recip reads r before bcast overwrites (sem-ordered)
    conv_ps = [ps([128, 512]) for _ in range(2)]

    vpad3 = [vp.rearrange("p (h w) -> p h w", h=PADH) for vp in vpad_sb]

    marks = {}  # name -> counter value (producer engine count at completion)

    # conv-bank users in PE order: list of keys; user i uses bank i%2 and
    # must WAR-wait on evac of user i-2 (marks['ev_'+key]).
    conv_users = []
    for m in range(4):
        for n in range(8):
            conv_users.append(f"qk_{m}_{n}")
    for t in range(32):
        conv_users.append(f"vt_{t}")
    for m in range(2):
        for n in range(8):
            conv_users.append(f"v_{m}_{n}")
    # dw tile u interleaved within unit u; proj tiles after each area
    seq = []
    for u, (a, hg, qt) in enumerate(UNITS):
        seq.append(f"dw_{u}")
        if hg == 1 and qt == 1:
            for nt in range(2):
                for m in range(2):
                    seq.append(f"pj_{2 * a + nt}_{m}")
    conv_users.extend(seq)
    user_bank = {k: i % 2 for i, k in enumerate(conv_users)}
    user_prev = {k: (conv_users[i - 2] if i >= 2 else None)
                 for i, k in enumerate(conv_users)}

    class Eng:
        """Counts instructions; in real mode also emits via `fns`."""

        def __init__(self, name, sem_name):
            self.name = name
            self.sem_name = sem_name
            self.n = 0

        def bump(self, binst, real, sems):
            self.n += 1
            if real:
                binst.then_inc(sems[self.sem_name], 1)

    def program(real, engines, sems):
        pe, act, dve, sp = engines["pe"], engines["act"], engines["dve"], engines["sp"]

        def w(eng_handle, sem_name, val):
            # standalone wait_ge; no count
            if real and val is not None and val > 0:
                eng_handle.wait_ge(sems[sem_name], val)

        def mark(name, eng):
            if not real:
                marks[name] = eng.n

        def get(name):
            return marks.get(name, 0)

        # ---------------- SP ----------------
        def sp_prog(h):
            n_dma = 0
            if real:
                for i in range(2):
                    h.dma_start(out=wqkT_sb[i], in_=wqkT_d[i * 128:(i + 1) * 128, :]).then_inc(sems["dma_i"], 16)
                    h.dma_start(out=wvT_sb[i], in_=wvT_d[i * 128:(i + 1) * 128, :]).then_inc(sems["dma_i"], 16)
                    h.dma_start(out=wprojT_sb[i], in_=wprojT_d[i * 128:(i + 1) * 128, :]).then_inc(sems["dma_i"], 16)
                    h.dma_start(out=dw_sb[i], in_=dwdiag_d[i, :, :]).then_inc(sems["dma_i"], 16)
                h.dma_start(out=ones_sb, in_=ones_d[:, :]).then_inc(sems["dma_i"], 16)
                h.dma_start(out=brows_sb, in_=brows_d[:, :]).then_inc(sems["dma_i"], 16)
                for i in range(2):
                    h.dma_start(out=x_sb[i], in_=x_d[i * 128:(i + 1) * 128, :]).then_inc(sems["dma_i"], 16)
            n_dma = 12
            assert n_dma == N_IN_DMA
            # out DMAs
            for k in range(8):            # token tile n
                for m in range(2):
                    idx = k * 2 + m
                    if real:
                        h.wait_ge(sems["dve_s"], marks[f"po_{k}_{m}"])
                        h.dma_start(out=out_d[m * 128:(m + 1) * 128, k * 512:(k + 1) * 512],
                                    in_=out_sb[idx]).then_inc(sems["dma_o"], 16)
            if real:
                h.wait_ge(sems["dma_o"], 16 * 16)

        # ---------------- PE ----------------
        def pe_prog(h):
            def mm(out, lhsT, rhs, start, stop, tp=None, inc=False):
                if real:
                    i = nc.tensor.matmul(out, lhsT, rhs, start=start, stop=stop,
                                         tile_position=tp, skip_group_check=True)
                    if inc:
                        pe.bump(i, real, sems)
                elif inc:
                    pe.n += 1

            def conv_war(key):
                prev = user_prev[key]
                if prev is not None:
                    w(h, "dve_s", get(f"ev_{prev}"))

            w(h, "dma_i", N_IN_DMA * 16)
            # qk GEMM
            for m in range(4):
                for n in range(8):
                    key = f"qk_{m}_{n}"
                    b = user_bank[key]
                    conv_war(key)
                    mm(conv_ps[b], wqkT_sb[0][:, m * 128:(m + 1) * 128],
                       x_sb[0][:, n * 512:(n + 1) * 512], True, False)
                    mm(conv_ps[b], wqkT_sb[1][:, m * 128:(m + 1) * 128],
                       x_sb[1][:, n * 512:(n + 1) * 512], False, False)
                    mm(conv_ps[b], brows_sb[0:1, m * 128:(m + 1) * 128],
                       ones_sb[0:1, 0:512], False, True, inc=True)
                    mark(key, pe)
            # vT GEMM
            for t in range(32):
                key = f"vt_{t}"
                b = user_bank[key]
                conv_war(key)
                mm(conv_ps[b][:, 0:256], x_sb[0][:, t * 128:(t + 1) * 128], wvT_sb[0], True, False)
                mm(conv_ps[b][:, 0:256], x_sb[1][:, t * 128:(t + 1) * 128], wvT_sb[1], False, True, inc=True)
                mark(key, pe)
            # v GEMM
            for m in range(2):
                for n in range(8):
                    key = f"v_{m}_{n}"
                    b = user_bank[key]
                    conv_war(key)
                    mm(conv_ps[b], wvT_sb[0][:, m * 128:(m + 1) * 128],
                       x_sb[0][:, n * 512:(n + 1) * 512], True, False)
                    mm(conv_ps[b], wvT_sb[1][:, m * 128:(m + 1) * 128],
                       x_sb[1][:, n * 512:(n + 1) * 512], False, True, inc=True)
                    mark(key, pe)
            # wait all startup evacs (qk ready for scores, vT for PV, vpad for dw)
            w(h, "dve_s", get("startup_evac"))

            for u, (a, hg, qt) in enumerate(UNITS):
                qb = a * NA + qt * 512

                def scores(kc):
                    kb = a * NA + kc * 128
                    for j in range(4):
                        mm(sc_ps[:, j * 512:(j + 1) * 512],
                           qk_sb[2 + hg][32 * j:32 * j + 32, kb:kb + 128],
                           qk_sb[hg][32 * j:32 * j + 32, qb:qb + 512],
                           True, True, tp=(32 * j, 0), inc=(j == 3))

                def esj_of(kc, j):
                    if j < 3:
                        return es_sb[kc % 2][:, j * 512:(j + 1) * 512]
                    return esd_sb[kc % 2][:, :]

                def pv_part(kc, js, inc_last):
                    tvt = a * 8 + kc
                    for j in js:
                        mm(u_ps[32 * j:32 * j + 32, :],
                           vT_sb[:, tvt * 256 + hg * 128 + 32 * j:
                                 tvt * 256 + hg * 128 + 32 * j + 32],
                           esj_of(kc, j), kc == 0, kc == 7, tp=(0, 32 * j))
                    for j in js:
                        mm(r_ps[32 * j:32 * j + 1, :], ones_sb[:, 0:1],
                           esj_of(kc, j), kc == 0, kc == 7, tp=(0, 32 * j),
                           inc=(inc_last and j == js[-1]))

                # dw filler chunks for dw tile u
                ct, dn = DW_TILES[u]
                dwkey = f"dw_{u}"
                dwb = user_bank[dwkey]

                def dw_chunk(ci):
                    taps = range((25 * ci) // 8, (25 * (ci + 1)) // 8)
                    for tap in taps:
                        if tap == 0:
                            conv_war(dwkey)
                        dy, dx = divmod(tap, 5)
                        mm(conv_ps[dwb], dw_sb[ct][:, tap * 128:(tap + 1) * 128],
                           vpad3[ct][:, 8 * dn + dy:8 * dn + dy + 8, dx:dx + WW],
                           tap == 0, tap == 24, inc=(tap == 24))
                    if taps and max(taps) == 24:
                        mark(dwkey, pe)

                # unit prologue: WAR on U/r/rb banks vs previous unit's DVE reads
                if u > 0:
                    w(h, "dve_s", get(f"unit_dve_{u - 1}"))
                scores(0)
                mark(f"grp_{u}_0", pe)
                for kc in range(1, 9):
                    dw_chunk(kc - 1)
                    w(h, "act_s", get(f"exp_{u}_{kc - 1}"))
                    pv_part(kc - 1, [0, 1, 2], False)
                    w(h, "dve_s", get(f"expd_{u}_{kc - 1}"))
                    pv_part(kc - 1, [3], True)
                    if kc < 8:
                        scores(kc)
                        mark(f"grp_{u}_{kc}", pe)
                mark(f"unitpv_{u}", pe)
                # rb broadcast (needs recip on DVE)
                w(h, "dve_s", get(f"recip_{u}"))
                for j in range(4):
                    mm(rb_ps[32 * j:32 * j + 32, :],
                       ones_sb[32 * j:32 * j + 1, 0:32],
                       rcinv_sb[32 * j:32 * j + 1, :],
                       True, True, tp=(32 * j, 32 * j), inc=(j == 3))
                mark(f"rb_{u}", pe)

                if hg == 1 and qt == 1:
                    # proj for area a
                    w(h, "dve_s", get(f"z_{a}"))
                    for nt in range(2):
                        k = 2 * a + nt
                        for m in range(2):
                            key = f"pj_{k}_{m}"
                            b = user_bank[key]
                            conv_war(key)
                            mm(conv_ps[b], wprojT_sb[0][:, m * 128:(m + 1) * 128],
                               z_sb[0][:, k * 512:(k + 1) * 512], True, False)
                            mm(conv_ps[b], wprojT_sb[1][:, m * 128:(m + 1) * 128],
                               z_sb[1][:, k * 512:(k + 1) * 512], False, False)
                            mm(conv_ps[b], brows_sb[0:1, 512 + m * 128:512 + (m + 1) * 128],
                               ones_sb[0:1, 0:512], False, True, inc=True)
                            mark(key, pe)

        # ---------------- ACT ----------------
        def act_prog(h):
            def ex(out, in_):
                if real:
                    i = nc.scalar.activation(out, in_, AF.Exp)
                    act.bump(i, real, sems)
                else:
                    act.n += 1

            for u, (a, hg, qt) in enumerate(UNITS):
                for kc in range(8):
                    w(h, "pe_s", get(f"grp_{u}_{kc}"))
                    ex(es_sb[kc % 2], sc_ps[:, 0:1536])
                    mark(f"exp_{u}_{kc}", act)

        # ---------------- DVE ----------------
        def dve_prog(h):
            def selfwait():
                if real:
                    h.wait_ge(sems["dve_s"], dve.n)

            def op(fn, *args, **kw):
                if real:
                    i = fn(*args, **kw)
                    dve.bump(i, real, sems)
                else:
                    dve.n += 1

            op(nc.vector.memset, vpad_sb[0], 0.0)
            op(nc.vector.memset, vpad_sb[1], 0.0)
            op(nc.vector.memset, r_ps, 1.0)

            for m in range(4):
                for n in range(8):
                    key = f"qk_{m}_{n}"
                    w(h, "pe_s", get(key))
                    op(nc.vector.tensor_copy,
                       qk_sb[m][:, n * 512:(n + 1) * 512], conv_ps[user_bank[key]])
                    mark(f"ev_{key}", dve)
            for t in range(32):
                key = f"vt_{t}"
                w(h, "pe_s", get(key))
                op(nc.vector.tensor_copy,
                   vT_sb[:, t * 256:(t + 1) * 256], conv_ps[user_bank[key]][:, 0:256])
                mark(f"ev_{key}", dve)
            selfwait()
            for m in range(2):
                for n in range(8):
                    key = f"v_{m}_{n}"
                    w(h, "pe_s", get(key))
                    op(nc.vector.tensor_copy,
                       vpad3[m][:, 2 + 8 * n:2 + 8 * n + 8, 2:2 + WW],
                       conv_ps[user_bank[key]].rearrange("p (r w) -> p r w", r=8))
                    mark(f"ev_{key}", dve)
            mark("startup_evac", dve)

            for u, (a, hg, qt) in enumerate(UNITS):
                qb = a * NA + qt * 512
                # head-3 exp via Schraudolph bit trick (single-bank PSUM read,
                # whole-tensor bitcast)
                for kc in range(8):
                    w(h, "pe_s", get(f"grp_{u}_{kc}"))
                    op(nc.vector.tensor_scalar,
                       esd_sb[kc % 2].bitcast(I16), sc_ps[:, 1536:2048],
                       SCH_K, SCH_B, ALU.mult, ALU.add)
                    mark(f"expd_{u}_{kc}", dve)
                w(h, "pe_s", get(f"unitpv_{u}"))
                selfwait()
                if real:
                    lp = nc.allow_low_precision("softmax denom to bf16")
                    lp.__enter__()
                op(nc.vector.reciprocal, rcinv_sb, r_ps)
                if real:
                    lp.__exit__(None, None, None)
                mark(f"recip_{u}", dve)
                op(nc.vector.tensor_copy, ucp_sb, u_ps)
                w(h, "pe_s", get(f"rb_{u}"))
                selfwait()
                op(nc.vector.tensor_mul,
                   oT_sb[hg][:, qb:qb + 512], ucp_sb, rb_ps)
                mark(f"unit_dve_{u}", dve)

                # dw evac for tile u
                ct, dn = DW_TILES[u]
                dwkey = f"dw_{u}"
                w(h, "pe_s", get(dwkey))
                op(nc.vector.tensor_copy,
                   pp_sb[ct][:, dn * 512:(dn + 1) * 512], conv_ps[user_bank[dwkey]])
                mark(f"ev_{dwkey}", dve)

                if hg == 1 and qt == 1:
                    # z = o + pp for area a
                    selfwait()
                    for cti in range(2):
                        op(nc.vector.tensor_add,
                           z_sb[cti][:, a * NA:(a + 1) * NA],
                           oT_sb[cti][:, a * NA:(a + 1) * NA],
                           pp_sb[cti][:, a * NA:(a + 1) * NA])
                    mark(f"z_{a}", dve)
                    # proj evacs
                    for nt in range(2):
                        k = 2 * a + nt
                        for m in range(2):
                            key = f"pj_{k}_{m}"
                            idx = k * 2 + m
                            w(h, "pe_s", get(key))
                            op(nc.vector.tensor_copy, out_sb[idx],
                               conv_ps[user_bank[key]])
                            mark(f"ev_{key}", dve)
                            mark(f"po_{k}_{m}", dve)

        if real:
            with nc.Block() as block, \
                 nc.semaphore("dma_i") as s_dma_i, \
                 nc.semaphore("dma_o") as s_dma_o, \
                 nc.semaphore("pe_s") as s_pe, \
                 nc.semaphore("act_s") as s_act, \
                 nc.semaphore("dve_s") as s_dve:
                sems.update({"dma_i": s_dma_i, "dma_o": s_dma_o,
                             "pe_s": s_pe, "act_s": s_act, "dve_s": s_dve})

                @block.sync
                def _(sync):
                    sp_prog(sync)

                @block.tensor
                def _(tensor):
                    pe_prog(tensor)

                @block.scalar
                def _(scalar):
                    act_prog(scalar)

                @block.vector
                def _(vector):
                    dve_prog(vector)
        else:
            class H:  # dry handle
                def wait_ge(self, *a, **k):
                    pass

                def dma_start(self, *a, **k):
                    class R:
                        def then_inc(self, *a, **k):
                            return self
                    return R()
            hh = H()
            sp_prog(hh)
            pe_prog(hh)
            act_prog(hh)
            dve_prog(hh)

    engines = {"pe": Eng("pe", "pe_s"), "act": Eng("act", "act_s"),
               "dve": Eng("dve", "dve_s"), "sp": Eng("sp", "dma_i")}
    sems = {}
    program(False, engines, sems)          # dry: fill marks
    engines = {"pe": Eng("pe", "pe_s"), "act": Eng("act", "act_s"),
               "dve": Eng("dve", "dve_s"), "sp": Eng("sp", "dma_i")}
    program(True, engines, sems)           # real emission
    return nc


@functools.lru_cache(maxsize=1)
def _get_nc():
    return _build_nc()


def _prep_host(inputs):
    x = np.asarray(inputs["x"], np.float32)            # [8, 256, 64, 64]
    w_qk = np.asarray(inputs["w_qk"], np.float32)      # [512, 256]
    s_qk = np.asarray(inputs["s_qk"], np.float32)
    b_qk = np.asarray(inputs["b_qk"], np.float32)
    w_v = np.asarray(inputs["w_v"], np.float32)
    s_v = np.asarray(inputs["s_v"], np.float32)
    b_v = np.asarray(inputs["b_v"], np.float32)
    w_pe = np.asarray(inputs["w_pe"], np.float32)      # [256, 1, 5, 5]
    s_pe = np.asarray(inputs["s_pe"], np.float32)
    b_pe = np.asarray(inputs["b_pe"], np.float32)
    w_proj = np.asarray(inputs["w_proj"], np.float32)
    s_proj = np.asarray(inputs["s_proj"], np.float32)
    b_proj = np.asarray(inputs["b_proj"], np.float32)

    # fold BN scales into weights; fold 1/sqrt(d) into q weights+bias
    w_qk_eff = w_qk * s_qk[:, None]
    b_qk_eff = b_qk * s_qk  # BN affine: y = s*(Wx) + b ... b is already the bias
    # NB: reference _conv1x1 computes  y = (Wx)*s + b, so bias is NOT scaled by s.
    b_qk_eff = b_qk.copy()
    w_qk_eff[:C] *= SCALE
    b_qk_eff[:C] *= SCALE

    w_v_eff = w_v * s_v[:, None]
    w_proj_eff = w_proj * s_proj[:, None]

    wpe = w_pe.reshape(C, 25)                          # [c, tap]
    wpe_eff = wpe * s_pe[:, None]

    # constants folded through attention/depthwise into proj bias:
    # o gets +b_v exactly (softmax rows sum to 1);
    # pp = s_pe*dw(v_nb) + s_pe*b_v*sum_taps(w_pe) + b_pe
    kappa = b_v + s_pe * b_v * wpe.sum(1) + b_pe       # [256]
    b_proj_eff = b_proj + w_proj_eff @ kappa

    dwdiag = np.zeros((2, 128, 25 * 128), np.float32)
    for ct in range(2):
        for tap in range(25):
            idx = np.arange(128)
            dwdiag[ct, idx, tap * 128 + idx] = wpe_eff[ct * 128 + idx, tap]

    common = {
        "wqkT": np.ascontiguousarray(w_qk_eff.T).astype(BF16NP),

        "wvT": np.ascontiguousarray(w_v_eff.T).astype(BF16NP),
        "wprojT": np.ascontiguousarray(w_proj_eff.T).astype(BF16NP),
        "brows": np.concatenate([b_qk_eff, b_proj_eff, np.zeros(256, np.float32)]
                                ).reshape(1, 1024).astype(BF16NP),
        "dwdiag": dwdiag.astype(BF16NP),
        "ones": np.ones((128, 512), BF16NP),
    }
    in_maps = []
    for i in range(8):
        m = dict(common)
        m["x"] = np.ascontiguousarray(x[i].reshape(C, NTOK)).astype(BF16NP)
        in_maps.append(m)
    return in_maps


def kernel(**inputs):
    nc = _get_nc()
    in_maps = _prep_host(inputs)
    res = run_bass_kernel_spmd(nc, in_maps, core_ids=list(range(8)))
    outs = [res.results[i]["out"].reshape(C, HH, WW) for i in range(8)]
    return np.stack(outs, 0).astype(np.float32)


if __name__ == "__main__":
    rng = np.random.default_rng(0)
    fake = {
        "x": rng.standard_normal((8, C, HH, WW), np.float32),
        "w_qk": rng.standard_normal((2 * C, C), np.float32) * 0.05,
        "s_qk": np.ones(2 * C, np.float32),
        "b_qk": rng.standard_normal(2 * C).astype(np.float32) * 0.01,
        "w_v": rng.standard_normal((C, C), np.float32) * 0.05,
        "s_v": np.ones(C, np.float32),
        "b_v": rng.standard_normal(C).astype(np.float32) * 0.01,
        "w_pe": rng.standard_normal((C, 1, 5, 5), np.float32) * 0.05,
        "s_pe": np.ones(C, np.float32),
        "b_pe": rng.standard_normal(C).astype(np.float32) * 0.01,
        "w_proj": rng.standard_normal((C, C), np.float32) * 0.05,
        "s_proj": np.ones(C, np.float32),
        "b_proj": rng.standard_normal(C).astype(np.float32) * 0.01,
    }
    out = kernel(**fake)
    print("out", out.shape, out.dtype, float(np.abs(out).mean()))



# revision 21
# speedup vs baseline: 1.4499x; 1.0206x over previous
"""Trainium2 Bass kernel for area-attention (YOLOv12 A2-style) module.

Raw-bass SPMD: 8 cores, 1 image each, no collectives. Engine split:
SP=DMA, PE=all matmuls (convs, dw-diag conv, attention with PE-array
row/col packing), ACT=softmax exp, DVE=evac/normalize/memset.
Software-pipelined per (area, head-group, q-tile) unit: exp(kc)
overlaps PV(kc-1)+scores(kc)+depthwise filler matmuls.
"""

import sys, os, functools

sys.path.insert(0, "/opt/trn_rl_repo")

import numpy as np
import ml_dtypes

import concourse.bass as bass
import concourse.mybir as mybir
from concourse.bass_utils import run_bass_kernel_spmd

BF16NP = ml_dtypes.bfloat16
F32 = mybir.dt.float32
BF = mybir.dt.bfloat16
I16 = mybir.dt.int16
ALU = mybir.AluOpType
SCH_K = 128.0 * float(np.log2(np.e))   # fold log2e into the DVE affine
SCH_B = 16248.636                      # mean-centered Schraudolph bias

C = 256
HH = 64
WW = 64
NTOK = HH * WW          # 4096
AREA = 4
NA = NTOK // AREA       # 1024
D = 32                  # head dim
SCALE = float(D) ** -0.5
PADW = WW + 4           # 68
PADH = HH + 4           # 68
NPAD = PADW * PADH      # 4624
AF = mybir.ActivationFunctionType

UNITS = [(a, hg, qt) for a in range(AREA) for hg in range(2) for qt in range(2)]
# dw output tile needed by proj of area a: (ct, 2a+qt)
DW_TILES = [(hg, 2 * a + qt) for a in range(AREA) for hg in range(2) for qt in range(2)]
N_IN_DMA = 12


def _build_nc():
    import contextlib
    nc = bass.Bass()

    x_d = nc.declare_dram_parameter("x", [C, NTOK], BF, isOutput=False)
    wqkT_d = nc.declare_dram_parameter("wqkT", [C, 2 * C], BF, isOutput=False)
    wvT_d = nc.declare_dram_parameter("wvT", [C, C], BF, isOutput=False)
    wprojT_d = nc.declare_dram_parameter("wprojT", [C, C], BF, isOutput=False)
    dwdiag_d = nc.declare_dram_parameter("dwdiag", [2, 128, 25 * 128], BF, isOutput=False)
    ones_d = nc.declare_dram_parameter("ones", [128, 512], BF, isOutput=False)
    bias_d = nc.declare_dram_parameter("bias", [128, 8], F32, isOutput=False)
    out_d = nc.declare_dram_parameter("out", [C, NTOK], F32, isOutput=True)

    ctx = contextlib.ExitStack()
    _names = [0]

    def sb(shape, dtype, nm=None):
        _names[0] += 1
        return ctx.enter_context(nc.sbuf_tensor(f"sb{_names[0]}", shape, dtype))[:, :]

    def ps(shape):
        _names[0] += 1
        return ctx.enter_context(nc.psum_tensor(f"ps{_names[0]}", shape, F32))[:, :]

    x_sb = [sb([128, NTOK], BF) for _ in range(2)]
    qk_sb = [sb([128, NTOK], BF) for _ in range(4)]
    vT_sb = sb([128, 32 * 256], BF)
    vpad_sb = [sb([128, NPAD], BF) for _ in range(2)]
    pp_sb = [sb([128, NTOK], BF) for _ in range(2)]
    oT_sb = [sb([128, NTOK], BF) for _ in range(2)]
    z_sb = [sb([128, NTOK], BF) for _ in range(2)]
    wqkT_sb = [sb([128, 2 * C], BF) for _ in range(2)]
    wvT_sb = [sb([128, C], BF) for _ in range(2)]
    wprojT_sb = [sb([128, C], BF) for _ in range(2)]
    dw_sb = [sb([128, 25 * 128], BF) for _ in range(2)]
    ones_sb = sb([128, 512], BF)
    bias_sb = sb([128, 8], F32)
    rcinv_sb = sb([128, 512], BF)
    es_sb = [sb([128, 1536], BF) for _ in range(2)]   # heads 0-2 (ACT exp)
    esd_sb = [sb([128, 512], BF) for _ in range(2)]   # head 3 (DVE Schraudolph)
    ucp_sb = sb([128, 512], F32)
    out_sb = [sb([128, 512], F32) for _ in range(16)]

    sc_ps = ps([128, 2048])   # 4 banks
    u_ps = ps([128, 512])
    r_ps = ps([128, 512])
    rb_ps = r_ps  # shared bank: recip reads r before bcast overwrites (sem-ordered)
    conv_ps = [ps([128, 512]) for _ in range(2)]

    vpad3 = [vp.rearrange("p (h w) -> p h w", h=PADH) for vp in vpad_sb]

    marks = {}  # name -> counter value (producer engine count at completion)

    # conv-bank users in PE order: list of keys; user i uses bank i%2 and
    # must WAR-wait on evac of user i-2 (marks['ev_'+key]).
    conv_users = []
    for m in range(4):
        for n in range(8):
            conv_users.append(f"qk_{m}_{n}")
    for t in range(32):
        conv_users.append(f"vt_{t}")
    for m in range(2):
        for n in range(8):
            conv_users.append(f"v_{m}_{n}")
    # dw tile u interleaved within unit u; proj tiles after each area
    seq = []
    for u, (a, hg, qt) in enumerate(UNITS):
        seq.append(f"dw_{u}")
        if hg == 1 and qt == 1:
            for nt in range(2):
                for m in range(2):
                    seq.append(f"pj_{2 * a + nt}_{m}")
    conv_users.extend(seq)
    user_bank = {k: i % 2 for i, k in enumerate(conv_users)}
    user_prev = {k: (conv_users[i - 2] if i >= 2 else None)
                 for i, k in enumerate(conv_users)}

    class Eng:
        """Counts instructions; in real mode also emits via `fns`."""

        def __init__(self, name, sem_name):
            self.name = name
            self.sem_name = sem_name
            self.n = 0

        def bump(self, binst, real, sems):
            self.n += 1
            if real:
                binst.then_inc(sems[self.sem_name], 1)

    def program(real, engines, sems):
        pe, act, dve, sp = engines["pe"], engines["act"], engines["dve"], engines["sp"]

        def w(eng_handle, sem_name, val):
            # standalone wait_ge; no count
            if real and val is not None and val > 0:
                eng_handle.wait_ge(sems[sem_name], val)

        def mark(name, eng):
            if not real:
                marks[name] = eng.n

        def get(name):
            return marks.get(name, 0)

        # ---------------- SP ----------------
        def sp_prog(h):
            n_dma = 0
            if real:
                for i in range(2):
                    h.dma_start(out=wqkT_sb[i], in_=wqkT_d[i * 128:(i + 1) * 128, :]).then_inc(sems["dma_i"], 16)
                    h.dma_start(out=wvT_sb[i], in_=wvT_d[i * 128:(i + 1) * 128, :]).then_inc(sems["dma_i"], 16)
                    h.dma_start(out=wprojT_sb[i], in_=wprojT_d[i * 128:(i + 1) * 128, :]).then_inc(sems["dma_i"], 16)
                    h.dma_start(out=dw_sb[i], in_=dwdiag_d[i, :, :]).then_inc(sems["dma_i"], 16)
                h.dma_start(out=ones_sb, in_=ones_d[:, :]).then_inc(sems["dma_i"], 16)
                h.dma_start(out=bias_sb, in_=bias_d[:, :]).then_inc(sems["dma_i"], 16)
                for i in range(2):
                    h.dma_start(out=x_sb[i], in_=x_d[i * 128:(i + 1) * 128, :]).then_inc(sems["dma_i"], 16)
            n_dma = 12
            assert n_dma == N_IN_DMA
            # out DMAs
            for k in range(8):            # token tile n
                for m in range(2):
                    idx = k * 2 + m
                    if real:
                        h.wait_ge(sems["dve_s"], marks[f"po_{k}_{m}"])
                        h.dma_start(out=out_d[m * 128:(m + 1) * 128, k * 512:(k + 1) * 512],
                                    in_=out_sb[idx]).then_inc(sems["dma_o"], 16)
            if real:
                h.wait_ge(sems["dma_o"], 16 * 16)

        # ---------------- PE ----------------
        def pe_prog(h):
            def mm(out, lhsT, rhs, start, stop, tp=None, inc=False):
                if real:
                    i = nc.tensor.matmul(out, lhsT, rhs, start=start, stop=stop,
                                         tile_position=tp, skip_group_check=True)
                    if inc:
                        pe.bump(i, real, sems)
                elif inc:
                    pe.n += 1

            def conv_war(key):
                prev = user_prev[key]
                if prev is not None:
                    # vt/v evacs run on ACT, everything else on DVE
                    sem = "act_s" if prev.startswith(("vt_", "v_")) else "dve_s"
                    w(h, sem, get(f"ev_{prev}"))

            w(h, "dma_i", N_IN_DMA * 16)
            # qk GEMM
            for m in range(4):
                for n in range(8):
                    key = f"qk_{m}_{n}"
                    b = user_bank[key]
                    conv_war(key)
                    mm(conv_ps[b], wqkT_sb[0][:, m * 128:(m + 1) * 128],
                       x_sb[0][:, n * 512:(n + 1) * 512], True, False)
                    mm(conv_ps[b], wqkT_sb[1][:, m * 128:(m + 1) * 128],
                       x_sb[1][:, n * 512:(n + 1) * 512], False, True, inc=True)
                    mark(key, pe)
            # vT GEMM
            for t in range(32):
                key = f"vt_{t}"
                b = user_bank[key]
                conv_war(key)
                mm(conv_ps[b][:, 0:256], x_sb[0][:, t * 128:(t + 1) * 128], wvT_sb[0], True, False)
                mm(conv_ps[b][:, 0:256], x_sb[1][:, t * 128:(t + 1) * 128], wvT_sb[1], False, True, inc=True)
                mark(key, pe)
            # v GEMM
            for m in range(2):
                for n in range(8):
                    key = f"v_{m}_{n}"
                    b = user_bank[key]
                    conv_war(key)
                    mm(conv_ps[b], wvT_sb[0][:, m * 128:(m + 1) * 128],
                       x_sb[0][:, n * 512:(n + 1) * 512], True, False)
                    mm(conv_ps[b], wvT_sb[1][:, m * 128:(m + 1) * 128],
                       x_sb[1][:, n * 512:(n + 1) * 512], False, True, inc=True)
                    mark(key, pe)
            # wait all startup evacs (qk on DVE; vT/vpad on ACT)
            w(h, "dve_s", get("qk_evac_done"))
            w(h, "act_s", get("startup_evac"))

            for u, (a, hg, qt) in enumerate(UNITS):
                qb = a * NA + qt * 512

                def scores(kc):
                    kb = a * NA + kc * 128
                    for j in range(4):
                        mm(sc_ps[:, j * 512:(j + 1) * 512],
                           qk_sb[2 + hg][32 * j:32 * j + 32, kb:kb + 128],
                           qk_sb[hg][32 * j:32 * j + 32, qb:qb + 512],
                           True, True, tp=(32 * j, 0), inc=(j == 3))

                def esj_of(kc, j):
                    if j < 3:
                        return es_sb[kc % 2][:, j * 512:(j + 1) * 512]
                    return esd_sb[kc % 2][:, :]

                def pv_part(kc, js, inc_last):
                    tvt = a * 8 + kc
                    for j in js:
                        mm(u_ps[32 * j:32 * j + 32, :],
                           vT_sb[:, tvt * 256 + hg * 128 + 32 * j:
                                 tvt * 256 + hg * 128 + 32 * j + 32],
                           esj_of(kc, j), kc == 0, kc == 7, tp=(0, 32 * j))
                    for j in js:
                        mm(r_ps[32 * j:32 * j + 1, :], ones_sb[:, 0:1],
                           esj_of(kc, j), kc == 0, kc == 7, tp=(0, 32 * j),
                           inc=(inc_last and j == js[-1]))

                # dw filler chunks for dw tile u
                ct, dn = DW_TILES[u]
                dwkey = f"dw_{u}"
                dwb = user_bank[dwkey]

                def dw_chunk(ci):
                    taps = range((25 * ci) // 8, (25 * (ci + 1)) // 8)
                    for tap in taps:
                        if tap == 0:
                            conv_war(dwkey)
                        dy, dx = divmod(tap, 5)
                        mm(conv_ps[dwb], dw_sb[ct][:, tap * 128:(tap + 1) * 128],
                           vpad3[ct][:, 8 * dn + dy:8 * dn + dy + 8, dx:dx + WW],
                           tap == 0, tap == 24, inc=(tap == 24))
                    if taps and max(taps) == 24:
                        mark(dwkey, pe)

                # unit prologue: WAR on U/r/rb banks vs previous unit's DVE reads
                if u > 0:
                    w(h, "dve_s", get(f"unit_dve_{u - 1}"))
                scores(0)
                mark(f"grp_{u}_0", pe)
                for kc in range(1, 9):
                    dw_chunk(kc - 1)
                    w(h, "act_s", get(f"exp_{u}_{kc - 1}"))
                    pv_part(kc - 1, [0, 1, 2], False)
                    w(h, "dve_s", get(f"expd_{u}_{kc - 1}"))
                    pv_part(kc - 1, [3], True)
                    if kc < 8:
                        scores(kc)
                        mark(f"grp_{u}_{kc}", pe)
                mark(f"unitpv_{u}", pe)
                # rb broadcast (needs recip on DVE)
                w(h, "dve_s", get(f"recip_{u}"))
                for j in range(4):
                    mm(rb_ps[32 * j:32 * j + 32, :],
                       ones_sb[32 * j:32 * j + 1, 0:32],
                       rcinv_sb[32 * j:32 * j + 1, :],
                       True, True, tp=(32 * j, 32 * j), inc=(j == 3))
                mark(f"rb_{u}", pe)

                if hg == 1 and qt == 1:
                    # proj for area a
                    w(h, "dve_s", get(f"z_{a}"))
                    for nt in range(2):
                        k = 2 * a + nt
                        for m in range(2):
                            key = f"pj_{k}_{m}"
                            b = user_bank[key]
                            conv_war(key)
                            mm(conv_ps[b], wprojT_sb[0][:, m * 128:(m + 1) * 128],
                               z_sb[0][:, k * 512:(k + 1) * 512], True, False)
                            mm(conv_ps[b], wprojT_sb[1][:, m * 128:(m + 1) * 128],
                               z_sb[1][:, k * 512:(k + 1) * 512], False, True, inc=True)
                            mark(key, pe)

        # ---------------- ACT ----------------
        def act_prog(h):
            def ex(out, in_):
                if real:
                    i = nc.scalar.activation(out, in_, AF.Exp)
                    act.bump(i, real, sems)
                else:
                    act.n += 1

            def cop(out, in_):
                if real:
                    i = nc.scalar.activation(out, in_, AF.Copy)
                    act.bump(i, real, sems)
                else:
                    act.n += 1

            # vT / v->vpad evacs (frees DVE for qk evacs; order matches PE's
            # GEMM order qk->vt->v, so no wait cycles)
            for t in range(32):
                key = f"vt_{t}"
                w(h, "pe_s", get(key))
                cop(vT_sb[:, t * 256:(t + 1) * 256],
                    conv_ps[user_bank[key]][:, 0:256])
                mark(f"ev_{key}", act)
            w(h, "dve_s", get("memsets_done"))
            for m in range(2):
                for n in range(8):
                    key = f"v_{m}_{n}"
                    w(h, "pe_s", get(key))
                    cop(vpad3[m][:, 2 + 8 * n:2 + 8 * n + 8, 2:2 + WW],
                        conv_ps[user_bank[key]].rearrange("p (r w) -> p r w", r=8))
                    mark(f"ev_{key}", act)
            mark("startup_evac", act)

            for u, (a, hg, qt) in enumerate(UNITS):
                for kc in range(8):
                    w(h, "pe_s", get(f"grp_{u}_{kc}"))
                    ex(es_sb[kc % 2], sc_ps[:, 0:1536])
                    mark(f"exp_{u}_{kc}", act)

        # ---------------- DVE ----------------
        def dve_prog(h):
            def selfwait():
                if real:
                    h.wait_ge(sems["dve_s"], dve.n)

            def op(fn, *args, **kw):
                if real:
                    i = fn(*args, **kw)
                    dve.bump(i, real, sems)
                else:
                    dve.n += 1

            op(nc.vector.memset, vpad_sb[0], 0.0)
            op(nc.vector.memset, vpad_sb[1], 0.0)
            op(nc.vector.memset, r_ps, 1.0)
            mark("memsets_done", dve)

            # qk evac with bias add (kills the rank-1 bias matmuls)
            for m in range(4):
                for n in range(8):
                    key = f"qk_{m}_{n}"
                    w(h, "pe_s", get(key))
                    op(nc.vector.tensor_scalar,
                       qk_sb[m][:, n * 512:(n + 1) * 512],
                       conv_ps[user_bank[key]],
                       bias_sb[:, m:m + 1], None, ALU.add)
                    mark(f"ev_{key}", dve)
            mark("qk_evac_done", dve)

            for u, (a, hg, qt) in enumerate(UNITS):
                qb = a * NA + qt * 512
                # head-3 exp via Schraudolph bit trick (single-bank PSUM read,
                # whole-tensor bitcast)
                for kc in range(8):
                    w(h, "pe_s", get(f"grp_{u}_{kc}"))
                    op(nc.vector.tensor_scalar,
                       esd_sb[kc % 2].bitcast(I16), sc_ps[:, 1536:2048],
                       SCH_K, SCH_B, ALU.mult, ALU.add)
                    mark(f"expd_{u}_{kc}", dve)
                w(h, "pe_s", get(f"unitpv_{u}"))
                selfwait()
                if real:
                    lp = nc.allow_low_precision("softmax denom to bf16")
                    lp.__enter__()
                op(nc.vector.reciprocal, rcinv_sb, r_ps)
                if real:
                    lp.__exit__(None, None, None)
                mark(f"recip_{u}", dve)
                op(nc.vector.tensor_copy, ucp_sb, u_ps)
                w(h, "pe_s", get(f"rb_{u}"))
                selfwait()
                op(nc.vector.tensor_mul,
                   oT_sb[hg][:, qb:qb + 512], ucp_sb, rb_ps)
                mark(f"unit_dve_{u}", dve)

                # dw evac for tile u
                ct, dn = DW_TILES[u]
                dwkey = f"dw_{u}"
                w(h, "pe_s", get(dwkey))
                op(nc.vector.tensor_copy,
                   pp_sb[ct][:, dn * 512:(dn + 1) * 512], conv_ps[user_bank[dwkey]])
                mark(f"ev_{dwkey}", dve)

                if hg == 1 and qt == 1:
                    # z = o + pp for area a
                    selfwait()
                    for cti in range(2):
                        op(nc.vector.tensor_add,
                           z_sb[cti][:, a * NA:(a + 1) * NA],
                           oT_sb[cti][:, a * NA:(a + 1) * NA],
                           pp_sb[cti][:, a * NA:(a + 1) * NA])
                    mark(f"z_{a}", dve)
                    # proj evacs
                    for nt in range(2):
                        k = 2 * a + nt
                        for m in range(2):
                            key = f"pj_{k}_{m}"
                            idx = k * 2 + m
                            w(h, "pe_s", get(key))
                            op(nc.vector.tensor_scalar, out_sb[idx],
                               conv_ps[user_bank[key]],
                               bias_sb[:, 4 + m:5 + m], None, ALU.add)
                            mark(f"ev_{key}", dve)
                            mark(f"po_{k}_{m}", dve)

        if real:
            with nc.Block() as block, \
                 nc.semaphore("dma_i") as s_dma_i, \
                 nc.semaphore("dma_o") as s_dma_o, \
                 nc.semaphore("pe_s") as s_pe, \
                 nc.semaphore("act_s") as s_act, \
                 nc.semaphore("dve_s") as s_dve:
                sems.update({"dma_i": s_dma_i, "dma_o": s_dma_o,
                             "pe_s": s_pe, "act_s": s_act, "dve_s": s_dve})

                @block.sync
                def _(sync):
                    sp_prog(sync)

                @block.tensor
                def _(tensor):
                    pe_prog(tensor)

                @block.scalar
                def _(scalar):
                    act_prog(scalar)

                @block.vector
                def _(vector):
                    dve_prog(vector)
        else:
            class H:  # dry handle
                def wait_ge(self, *a, **k):
                    pass

                def dma_start(self, *a, **k):
                    class R:
                        def then_inc(self, *a, **k):
                            return self
                    return R()
            hh = H()
            sp_prog(hh)
            pe_prog(hh)
            act_prog(hh)
            dve_prog(hh)

    engines = {"pe": Eng("pe", "pe_s"), "act": Eng("act", "act_s"),
               "dve": Eng("dve", "dve_s"), "sp": Eng("sp", "dma_i")}
    sems = {}
    program(False, engines, sems)          # dry: fill marks
    engines = {"pe": Eng("pe", "pe_s"), "act": Eng("act", "act_s"),
               "dve": Eng("dve", "dve_s"), "sp": Eng("sp", "dma_i")}
    program(True, engines, sems)           # real emission
    return nc


@functools.lru_cache(maxsize=1)
def _get_nc():
    return _build_nc()


def _prep_host(inputs):
    x = np.asarray(inputs["x"], np.float32)            # [8, 256, 64, 64]
    w_qk = np.asarray(inputs["w_qk"], np.float32)      # [512, 256]
    s_qk = np.asarray(inputs["s_qk"], np.float32)
    b_qk = np.asarray(inputs["b_qk"], np.float32)
    w_v = np.asarray(inputs["w_v"], np.float32)
    s_v = np.asarray(inputs["s_v"], np.float32)
    b_v = np.asarray(inputs["b_v"], np.float32)
    w_pe = np.asarray(inputs["w_pe"], np.float32)      # [256, 1, 5, 5]
    s_pe = np.asarray(inputs["s_pe"], np.float32)
    b_pe = np.asarray(inputs["b_pe"], np.float32)
    w_proj = np.asarray(inputs["w_proj"], np.float32)
    s_proj = np.asarray(inputs["s_proj"], np.float32)
    b_proj = np.asarray(inputs["b_proj"], np.float32)

    # fold BN scales into weights; fold 1/sqrt(d) into q weights+bias
    w_qk_eff = w_qk * s_qk[:, None]
    b_qk_eff = b_qk * s_qk  # BN affine: y = s*(Wx) + b ... b is already the bias
    # NB: reference _conv1x1 computes  y = (Wx)*s + b, so bias is NOT scaled by s.
    b_qk_eff = b_qk.copy()
    w_qk_eff[:C] *= SCALE
    b_qk_eff[:C] *= SCALE

    w_v_eff = w_v * s_v[:, None]
    w_proj_eff = w_proj * s_proj[:, None]

    wpe = w_pe.reshape(C, 25)                          # [c, tap]
    wpe_eff = wpe * s_pe[:, None]

    # constants folded through attention/depthwise into proj bias:
    # o gets +b_v exactly (softmax rows sum to 1);
    # pp = s_pe*dw(v_nb) + s_pe*b_v*sum_taps(w_pe) + b_pe
    kappa = b_v + s_pe * b_v * wpe.sum(1) + b_pe       # [256]
    b_proj_eff = b_proj + w_proj_eff @ kappa

    dwdiag = np.zeros((2, 128, 25 * 128), np.float32)
    for ct in range(2):
        for tap in range(25):
            idx = np.arange(128)
            dwdiag[ct, idx, tap * 128 + idx] = wpe_eff[ct * 128 + idx, tap]

    common = {
        "wqkT": np.ascontiguousarray(w_qk_eff.T).astype(BF16NP),

        "wvT": np.ascontiguousarray(w_v_eff.T).astype(BF16NP),
        "wprojT": np.ascontiguousarray(w_proj_eff.T).astype(BF16NP),
        "bias": np.stack([b_qk_eff[0:128], b_qk_eff[128:256],
                          b_qk_eff[256:384], b_qk_eff[384:512],
                          b_proj_eff[0:128], b_proj_eff[128:256],
                          np.zeros(128, np.float32), np.zeros(128, np.float32)],
                         axis=1).astype(np.float32),
        "dwdiag": dwdiag.astype(BF16NP),
        "ones": np.ones((128, 512), BF16NP),
    }
    in_maps = []
    for i in range(8):
        m = dict(common)
        m["x"] = np.ascontiguousarray(x[i].reshape(C, NTOK)).astype(BF16NP)
        in_maps.append(m)
    return in_maps


def kernel(**inputs):
    nc = _get_nc()
    in_maps = _prep_host(inputs)
    res = run_bass_kernel_spmd(nc, in_maps, core_ids=list(range(8)))
    outs = [res.results[i]["out"].reshape(C, HH, WW) for i in range(8)]
    return np.stack(outs, 0).astype(np.float32)


if __name__ == "__main__":
    rng = np.random.default_rng(0)
    fake = {
        "x": rng.standard_normal((8, C, HH, WW), np.float32),
        "w_qk": rng.standard_normal((2 * C, C), np.float32) * 0.05,
        "s_qk": np.ones(2 * C, np.float32),
        "b_qk": rng.standard_normal(2 * C).astype(np.float32) * 0.01,
        "w_v": rng.standard_normal((C, C), np.float32) * 0.05,
        "s_v": np.ones(C, np.float32),
        "b_v": rng.standard_normal(C).astype(np.float32) * 0.01,
        "w_pe": rng.standard_normal((C, 1, 5, 5), np.float32) * 0.05,
        "s_pe": np.ones(C, np.float32),
        "b_pe": rng.standard_normal(C).astype(np.float32) * 0.01,
        "w_proj": rng.standard_normal((C, C), np.float32) * 0.05,
        "s_proj": np.ones(C, np.float32),
        "b_proj": rng.standard_normal(C).astype(np.float32) * 0.01,
    }
    out = kernel(**fake)
    print("out", out.shape, out.dtype, float(np.abs(out).mean()))



# revision 22
# speedup vs baseline: 1.6415x; 1.1321x over previous
"""Trainium2 Bass kernel for area-attention (YOLOv12 A2-style) module.

Raw-bass SPMD: 8 cores, 1 image each, no collectives. Engine split:
SP=DMA, PE=all matmuls (convs, dw-diag conv, attention with PE-array
row/col packing), ACT=softmax exp, DVE=evac/normalize/memset.
Software-pipelined per (area, head-group, q-tile) unit: exp(kc)
overlaps PV(kc-1)+scores(kc)+depthwise filler matmuls.
"""

import sys, os, functools

sys.path.insert(0, "/opt/trn_rl_repo")

import numpy as np
import ml_dtypes

import concourse.bass as bass
import concourse.mybir as mybir
from concourse.bass_utils import run_bass_kernel_spmd

BF16NP = ml_dtypes.bfloat16
F32 = mybir.dt.float32
BF = mybir.dt.bfloat16
I16 = mybir.dt.int16
ALU = mybir.AluOpType
SCH_K = 128.0 * float(np.log2(np.e))   # fold log2e into the DVE affine
SCH_B = 16248.636                      # mean-centered Schraudolph bias

C = 256
HH = 64
WW = 64
NTOK = HH * WW          # 4096
AREA = 4
NA = NTOK // AREA       # 1024
D = 32                  # head dim
SCALE = float(D) ** -0.5
PADW = WW + 4           # 68
PADH = HH + 4           # 68
NPAD = PADW * PADH      # 4624
AF = mybir.ActivationFunctionType

UNITS = [(a, hg, qt) for a in range(AREA) for hg in range(2) for qt in range(2)]
# dw output tile needed by proj of area a: (ct, 2a+qt)
DW_TILES = [(hg, 2 * a + qt) for a in range(AREA) for hg in range(2) for qt in range(2)]
N_IN_DMA = 12


def _build_nc():
    import contextlib
    nc = bass.Bass()

    x_d = nc.declare_dram_parameter("x", [C, NTOK], BF, isOutput=False)
    wqkT_d = nc.declare_dram_parameter("wqkT", [C, 2 * C], BF, isOutput=False)
    wvT_d = nc.declare_dram_parameter("wvT", [C, C], BF, isOutput=False)
    wprojT_d = nc.declare_dram_parameter("wprojT", [C, C], BF, isOutput=False)
    dwdiag_d = nc.declare_dram_parameter("dwdiag", [2, 128, 25 * 128], BF, isOutput=False)
    ones_d = nc.declare_dram_parameter("ones", [128, 512], BF, isOutput=False)
    bias_d = nc.declare_dram_parameter("bias", [128, 8], F32, isOutput=False)
    out_d = nc.declare_dram_parameter("out", [C, NTOK], F32, isOutput=True)

    ctx = contextlib.ExitStack()
    _names = [0]

    def sb(shape, dtype, nm=None):
        _names[0] += 1
        return ctx.enter_context(nc.sbuf_tensor(f"sb{_names[0]}", shape, dtype))[:, :]

    def ps(shape):
        _names[0] += 1
        return ctx.enter_context(nc.psum_tensor(f"ps{_names[0]}", shape, F32))[:, :]

    x_sb = [sb([128, NTOK], BF) for _ in range(2)]
    qk_sb = [sb([128, NTOK], BF) for _ in range(4)]
    vT_sb = sb([128, 32 * 256], BF)
    vpad_sb = [sb([128, NPAD], BF) for _ in range(2)]
    pp_sb = [sb([128, NTOK], BF) for _ in range(2)]
    oT_sb = [sb([128, NTOK], BF) for _ in range(2)]
    z_sb = [sb([128, NTOK], BF) for _ in range(2)]
    wqkT_sb = [sb([128, 2 * C], BF) for _ in range(2)]
    wvT_sb = [sb([128, C], BF) for _ in range(2)]
    wprojT_sb = [sb([128, C], BF) for _ in range(2)]
    dw_sb = [sb([128, 25 * 128], BF) for _ in range(2)]
    ones_sb = sb([128, 512], BF)
    bias_sb = sb([128, 8], F32)
    rcinv_sb = sb([128, 512], BF)
    es_sb = [sb([128, 1536], BF) for _ in range(2)]   # heads 0-2 (ACT exp)
    esd_sb = [sb([128, 512], BF) for _ in range(2)]   # head 3 (DVE Schraudolph)
    ucp_sb = sb([128, 512], F32)
    out_sb = [sb([128, 512], F32) for _ in range(16)]

    sc_ps = ps([128, 2048])   # 4 banks
    u_ps = ps([128, 512])
    r_ps = ps([128, 512])
    rb_ps = r_ps  # shared bank: recip reads r before bcast overwrites (sem-ordered)
    conv_ps = [ps([128, 512]) for _ in range(2)]

    vpad3 = [vp.rearrange("p (h w) -> p h w", h=PADH) for vp in vpad_sb]

    marks = {}  # name -> counter value (producer engine count at completion)

    # conv-bank users in PE order: list of keys; user i uses bank i%2 and
    # must WAR-wait on evac of user i-2 (marks['ev_'+key]).
    conv_users = []
    for m in range(4):
        for n in range(8):
            conv_users.append(f"qk_{m}_{n}")
    for t in range(32):
        conv_users.append(f"vt_{t}")
    for m in range(2):
        for n in range(8):
            conv_users.append(f"v_{m}_{n}")
    # dw tile u interleaved within unit u; proj tiles after each area
    seq = []
    for u, (a, hg, qt) in enumerate(UNITS):
        seq.append(f"dw_{u}")
        if hg == 1 and qt == 1:
            for nt in range(2):
                for m in range(2):
                    seq.append(f"pj_{2 * a + nt}_{m}")
    conv_users.extend(seq)
    user_bank = {k: i % 2 for i, k in enumerate(conv_users)}
    user_prev = {k: (conv_users[i - 2] if i >= 2 else None)
                 for i, k in enumerate(conv_users)}

    class Eng:
        """Counts instructions; in real mode also emits via `fns`."""

        def __init__(self, name, sem_name):
            self.name = name
            self.sem_name = sem_name
            self.n = 0

        def bump(self, binst, real, sems):
            self.n += 1
            if real:
                binst.then_inc(sems[self.sem_name], 1)

    def program(real, engines, sems):
        pe, act, dve, sp = engines["pe"], engines["act"], engines["dve"], engines["sp"]

        def w(eng_handle, sem_name, val):
            # standalone wait_ge; no count
            if real and val is not None and val > 0:
                eng_handle.wait_ge(sems[sem_name], val)

        def mark(name, eng):
            if not real:
                marks[name] = eng.n

        def get(name):
            return marks.get(name, 0)

        # ---------------- SP ----------------
        def sp_prog(h):
            n_dma = 0
            if real:
                for i in range(2):
                    h.dma_start(out=wqkT_sb[i], in_=wqkT_d[i * 128:(i + 1) * 128, :]).then_inc(sems["dma_i"], 16)
                    h.dma_start(out=wvT_sb[i], in_=wvT_d[i * 128:(i + 1) * 128, :]).then_inc(sems["dma_i"], 16)
                    h.dma_start(out=wprojT_sb[i], in_=wprojT_d[i * 128:(i + 1) * 128, :]).then_inc(sems["dma_i"], 16)
                    h.dma_start(out=dw_sb[i], in_=dwdiag_d[i, :, :]).then_inc(sems["dma_i"], 16)
                h.dma_start(out=ones_sb, in_=ones_d[:, :]).then_inc(sems["dma_i"], 16)
                h.dma_start(out=bias_sb, in_=bias_d[:, :]).then_inc(sems["dma_i"], 16)
                for i in range(2):
                    h.dma_start(out=x_sb[i], in_=x_d[i * 128:(i + 1) * 128, :]).then_inc(sems["dma_i"], 16)
            n_dma = 12
            assert n_dma == N_IN_DMA
            # out DMAs
            for k in range(8):            # token tile n
                for m in range(2):
                    idx = k * 2 + m
                    if real:
                        h.wait_ge(sems["dve_s"], marks[f"po_{k}_{m}"])
                        h.dma_start(out=out_d[m * 128:(m + 1) * 128, k * 512:(k + 1) * 512],
                                    in_=out_sb[idx]).then_inc(sems["dma_o"], 16)
            if real:
                h.wait_ge(sems["dma_o"], 16 * 16)

        # ---------------- PE ----------------
        def pe_prog(h):
            def mm(out, lhsT, rhs, start, stop, tp=None, inc=False):
                if real:
                    i = nc.tensor.matmul(out, lhsT, rhs, start=start, stop=stop,
                                         tile_position=tp, skip_group_check=True)
                    if inc:
                        pe.bump(i, real, sems)
                elif inc:
                    pe.n += 1

            def conv_war(key):
                prev = user_prev[key]
                if prev is not None:
                    # vt/v evacs run on ACT, everything else on DVE
                    sem = "act_s" if prev.startswith(("vt_", "v_")) else "dve_s"
                    w(h, sem, get(f"ev_{prev}"))

            w(h, "dma_i", N_IN_DMA * 16)
            # qk GEMM
            for m in range(4):
                for n in range(8):
                    key = f"qk_{m}_{n}"
                    b = user_bank[key]
                    conv_war(key)
                    mm(conv_ps[b], wqkT_sb[0][:, m * 128:(m + 1) * 128],
                       x_sb[0][:, n * 512:(n + 1) * 512], True, False)
                    mm(conv_ps[b], wqkT_sb[1][:, m * 128:(m + 1) * 128],
                       x_sb[1][:, n * 512:(n + 1) * 512], False, True, inc=True)
                    mark(key, pe)
            # vT GEMM
            for t in range(32):
                key = f"vt_{t}"
                b = user_bank[key]
                conv_war(key)
                mm(conv_ps[b][:, 0:256], x_sb[0][:, t * 128:(t + 1) * 128], wvT_sb[0], True, False)
                mm(conv_ps[b][:, 0:256], x_sb[1][:, t * 128:(t + 1) * 128], wvT_sb[1], False, True, inc=True)
                mark(key, pe)
            # v GEMM
            for m in range(2):
                for n in range(8):
                    key = f"v_{m}_{n}"
                    b = user_bank[key]
                    conv_war(key)
                    mm(conv_ps[b], wvT_sb[0][:, m * 128:(m + 1) * 128],
                       x_sb[0][:, n * 512:(n + 1) * 512], True, False)
                    mm(conv_ps[b], wvT_sb[1][:, m * 128:(m + 1) * 128],
                       x_sb[1][:, n * 512:(n + 1) * 512], False, True, inc=True)
                    mark(key, pe)
            # wait all startup evacs (qk on DVE; vT/vpad on ACT)
            w(h, "dve_s", get("qk_evac_done"))
            w(h, "act_s", get("startup_evac"))

            def scores_g(u2, kc):
                a2, hg2, qt2 = UNITS[u2]
                qb2 = a2 * NA + qt2 * 512
                kb = a2 * NA + kc * 128
                for j in range(4):
                    mm(sc_ps[:, j * 512:(j + 1) * 512],
                       qk_sb[2 + hg2][32 * j:32 * j + 32, kb:kb + 128],
                       qk_sb[hg2][32 * j:32 * j + 32, qb2:qb2 + 512],
                       True, True, tp=(32 * j, 0), inc=(j == 3))
                mark(f"grp_{u2}_{kc}", pe)

            for u, (a, hg, qt) in enumerate(UNITS):
                qb = a * NA + qt * 512

                def scores(kc):
                    scores_g(u, kc)

                def esj_of(kc, j):
                    if j < 3:
                        return es_sb[kc % 2][:, j * 512:(j + 1) * 512]
                    return esd_sb[kc % 2][:, :]

                def pv_part(kc, js, inc_last):
                    tvt = a * 8 + kc
                    for j in js:
                        mm(u_ps[32 * j:32 * j + 32, :],
                           vT_sb[:, tvt * 256 + hg * 128 + 32 * j:
                                 tvt * 256 + hg * 128 + 32 * j + 32],
                           esj_of(kc, j), kc == 0, kc == 7, tp=(0, 32 * j))
                    for j in js:
                        mm(r_ps[32 * j:32 * j + 1, :], ones_sb[:, 0:1],
                           esj_of(kc, j), kc == 0, kc == 7, tp=(0, 32 * j),
                           inc=(inc_last and j == js[-1]))

                # dw filler chunks for dw tile u
                ct, dn = DW_TILES[u]
                dwkey = f"dw_{u}"
                dwb = user_bank[dwkey]

                def dw_chunk(ci):
                    taps = range((25 * ci) // 8, (25 * (ci + 1)) // 8)
                    for tap in taps:
                        if tap == 0:
                            conv_war(dwkey)
                        dy, dx = divmod(tap, 5)
                        mm(conv_ps[dwb], dw_sb[ct][:, tap * 128:(tap + 1) * 128],
                           vpad3[ct][:, 8 * dn + dy:8 * dn + dy + 8, dx:dx + WW],
                           tap == 0, tap == 24, inc=(tap == 24))
                    if taps and max(taps) == 24:
                        mark(dwkey, pe)

                # unit prologue: WAR on U/r/rb banks vs previous unit's DVE
                # reads. scores(0) of unit 0 issues here; later units get
                # their scores(0) hoisted to the end of the previous unit.
                if u > 0:
                    w(h, "dve_s", get(f"unit_dve_{u - 1}"))
                else:
                    scores(0)
                for kc in range(1, 9):
                    dw_chunk(kc - 1)
                    w(h, "act_s", get(f"exp_{u}_{kc - 1}"))
                    pv_part(kc - 1, [0, 1, 2], False)
                    w(h, "dve_s", get(f"expd_{u}_{kc - 1}"))
                    pv_part(kc - 1, [3], True)
                    if kc < 8:
                        scores(kc)
                mark(f"unitpv_{u}", pe)
                # Hoist the NEXT unit's first scores ahead of the reciprocal
                # wait so ACT computes exp(u+1, 0) during the recip+rb+mul
                # window (sc banks are free: exp/expd(u,7) completed before
                # pv(7)).
                if u + 1 < len(UNITS):
                    scores_g(u + 1, 0)
                # rb broadcast (needs recip on DVE)
                w(h, "dve_s", get(f"recip_{u}"))
                for j in range(4):
                    mm(rb_ps[32 * j:32 * j + 32, :],
                       ones_sb[32 * j:32 * j + 1, 0:32],
                       rcinv_sb[32 * j:32 * j + 1, :],
                       True, True, tp=(32 * j, 32 * j), inc=(j == 3))
                mark(f"rb_{u}", pe)

                if hg == 1 and qt == 1:
                    # proj for area a
                    w(h, "dve_s", get(f"z_{a}"))
                    for nt in range(2):
                        k = 2 * a + nt
                        for m in range(2):
                            key = f"pj_{k}_{m}"
                            b = user_bank[key]
                            conv_war(key)
                            mm(conv_ps[b], wprojT_sb[0][:, m * 128:(m + 1) * 128],
                               z_sb[0][:, k * 512:(k + 1) * 512], True, False)
                            mm(conv_ps[b], wprojT_sb[1][:, m * 128:(m + 1) * 128],
                               z_sb[1][:, k * 512:(k + 1) * 512], False, True, inc=True)
                            mark(key, pe)

        # ---------------- ACT ----------------
        def act_prog(h):
            def ex(out, in_):
                if real:
                    i = nc.scalar.activation(out, in_, AF.Exp)
                    act.bump(i, real, sems)
                else:
                    act.n += 1

            def cop(out, in_):
                if real:
                    i = nc.scalar.activation(out, in_, AF.Copy)
                    act.bump(i, real, sems)
                else:
                    act.n += 1

            # vT / v->vpad evacs (frees DVE for qk evacs; order matches PE's
            # GEMM order qk->vt->v, so no wait cycles)
            for t in range(32):
                key = f"vt_{t}"
                w(h, "pe_s", get(key))
                cop(vT_sb[:, t * 256:(t + 1) * 256],
                    conv_ps[user_bank[key]][:, 0:256])
                mark(f"ev_{key}", act)
            w(h, "dve_s", get("memsets_done"))
            for m in range(2):
                for n in range(8):
                    key = f"v_{m}_{n}"
                    w(h, "pe_s", get(key))
                    cop(vpad3[m][:, 2 + 8 * n:2 + 8 * n + 8, 2:2 + WW],
                        conv_ps[user_bank[key]].rearrange("p (r w) -> p r w", r=8))
                    mark(f"ev_{key}", act)
            mark("startup_evac", act)

            for u, (a, hg, qt) in enumerate(UNITS):
                for kc in range(8):
                    w(h, "pe_s", get(f"grp_{u}_{kc}"))
                    ex(es_sb[kc % 2], sc_ps[:, 0:1536])
                    mark(f"exp_{u}_{kc}", act)

        # ---------------- DVE ----------------
        def dve_prog(h):
            def selfwait():
                if real:
                    h.wait_ge(sems["dve_s"], dve.n)

            def op(fn, *args, **kw):
                if real:
                    i = fn(*args, **kw)
                    dve.bump(i, real, sems)
                else:
                    dve.n += 1

            op(nc.vector.memset, vpad_sb[0], 0.0)
            op(nc.vector.memset, vpad_sb[1], 0.0)
            op(nc.vector.memset, r_ps, 1.0)
            mark("memsets_done", dve)

            # qk evac with bias add (kills the rank-1 bias matmuls)
            for m in range(4):
                for n in range(8):
                    key = f"qk_{m}_{n}"
                    w(h, "pe_s", get(key))
                    op(nc.vector.tensor_scalar,
                       qk_sb[m][:, n * 512:(n + 1) * 512],
                       conv_ps[user_bank[key]],
                       bias_sb[:, m:m + 1], None, ALU.add)
                    mark(f"ev_{key}", dve)
            mark("qk_evac_done", dve)

            for u, (a, hg, qt) in enumerate(UNITS):
                qb = a * NA + qt * 512
                # head-3 exp via Schraudolph bit trick (single-bank PSUM read,
                # whole-tensor bitcast)
                for kc in range(8):
                    w(h, "pe_s", get(f"grp_{u}_{kc}"))
                    op(nc.vector.tensor_scalar,
                       esd_sb[kc % 2].bitcast(I16), sc_ps[:, 1536:2048],
                       SCH_K, SCH_B, ALU.mult, ALU.add)
                    mark(f"expd_{u}_{kc}", dve)
                w(h, "pe_s", get(f"unitpv_{u}"))
                selfwait()
                if real:
                    lp = nc.allow_low_precision("softmax denom to bf16")
                    lp.__enter__()
                op(nc.vector.reciprocal, rcinv_sb, r_ps)
                if real:
                    lp.__exit__(None, None, None)
                mark(f"recip_{u}", dve)
                op(nc.vector.tensor_copy, ucp_sb, u_ps)
                w(h, "pe_s", get(f"rb_{u}"))
                selfwait()
                op(nc.vector.tensor_mul,
                   oT_sb[hg][:, qb:qb + 512], ucp_sb, rb_ps)
                mark(f"unit_dve_{u}", dve)

                # dw evac for tile u
                ct, dn = DW_TILES[u]
                dwkey = f"dw_{u}"
                w(h, "pe_s", get(dwkey))
                op(nc.vector.tensor_copy,
                   pp_sb[ct][:, dn * 512:(dn + 1) * 512], conv_ps[user_bank[dwkey]])
                mark(f"ev_{dwkey}", dve)

                if hg == 1 and qt == 1:
                    # z = o + pp for area a
                    selfwait()
                    for cti in range(2):
                        op(nc.vector.tensor_add,
                           z_sb[cti][:, a * NA:(a + 1) * NA],
                           oT_sb[cti][:, a * NA:(a + 1) * NA],
                           pp_sb[cti][:, a * NA:(a + 1) * NA])
                    mark(f"z_{a}", dve)
                    # proj evacs
                    for nt in range(2):
                        k = 2 * a + nt
                        for m in range(2):
                            key = f"pj_{k}_{m}"
                            idx = k * 2 + m
                            w(h, "pe_s", get(key))
                            op(nc.vector.tensor_scalar, out_sb[idx],
                               conv_ps[user_bank[key]],
                               bias_sb[:, 4 + m:5 + m], None, ALU.add)
                            mark(f"ev_{key}", dve)
                            mark(f"po_{k}_{m}", dve)

        if real:
            with nc.Block() as block, \
                 nc.semaphore("dma_i") as s_dma_i, \
                 nc.semaphore("dma_o") as s_dma_o, \
                 nc.semaphore("pe_s") as s_pe, \
                 nc.semaphore("act_s") as s_act, \
                 nc.semaphore("dve_s") as s_dve:
                sems.update({"dma_i": s_dma_i, "dma_o": s_dma_o,
                             "pe_s": s_pe, "act_s": s_act, "dve_s": s_dve})

                @block.sync
                def _(sync):
                    sp_prog(sync)

                @block.tensor
                def _(tensor):
                    pe_prog(tensor)

                @block.scalar
                def _(scalar):
                    act_prog(scalar)

                @block.vector
                def _(vector):
                    dve_prog(vector)
        else:
            class H:  # dry handle
                def wait_ge(self, *a, **k):
                    pass

                def dma_start(self, *a, **k):
                    class R:
                        def then_inc(self, *a, **k):
                            return self
                    return R()
            hh = H()
            sp_prog(hh)
            pe_prog(hh)
            act_prog(hh)
            dve_prog(hh)

    engines = {"pe": Eng("pe", "pe_s"), "act": Eng("act", "act_s"),
               "dve": Eng("dve", "dve_s"), "sp": Eng("sp", "dma_i")}
    sems = {}
    program(False, engines, sems)          # dry: fill marks
    engines = {"pe": Eng("pe", "pe_s"), "act": Eng("act", "act_s"),
               "dve": Eng("dve", "dve_s"), "sp": Eng("sp", "dma_i")}
    program(True, engines, sems)           # real emission
    return nc


@functools.lru_cache(maxsize=1)
def _get_nc():
    return _build_nc()


def _prep_host(inputs):
    x = np.asarray(inputs["x"], np.float32)            # [8, 256, 64, 64]
    w_qk = np.asarray(inputs["w_qk"], np.float32)      # [512, 256]
    s_qk = np.asarray(inputs["s_qk"], np.float32)
    b_qk = np.asarray(inputs["b_qk"], np.float32)
    w_v = np.asarray(inputs["w_v"], np.float32)
    s_v = np.asarray(inputs["s_v"], np.float32)
    b_v = np.asarray(inputs["b_v"], np.float32)
    w_pe = np.asarray(inputs["w_pe"], np.float32)      # [256, 1, 5, 5]
    s_pe = np.asarray(inputs["s_pe"], np.float32)
    b_pe = np.asarray(inputs["b_pe"], np.float32)
    w_proj = np.asarray(inputs["w_proj"], np.float32)
    s_proj = np.asarray(inputs["s_proj"], np.float32)
    b_proj = np.asarray(inputs["b_proj"], np.float32)

    # fold BN scales into weights; fold 1/sqrt(d) into q weights+bias
    w_qk_eff = w_qk * s_qk[:, None]
    b_qk_eff = b_qk * s_qk  # BN affine: y = s*(Wx) + b ... b is already the bias
    # NB: reference _conv1x1 computes  y = (Wx)*s + b, so bias is NOT scaled by s.
    b_qk_eff = b_qk.copy()
    w_qk_eff[:C] *= SCALE
    b_qk_eff[:C] *= SCALE

    w_v_eff = w_v * s_v[:, None]
    w_proj_eff = w_proj * s_proj[:, None]

    wpe = w_pe.reshape(C, 25)                          # [c, tap]
    wpe_eff = wpe * s_pe[:, None]

    # constants folded through attention/depthwise into proj bias:
    # o gets +b_v exactly (softmax rows sum to 1);
    # pp = s_pe*dw(v_nb) + s_pe*b_v*sum_taps(w_pe) + b_pe
    kappa = b_v + s_pe * b_v * wpe.sum(1) + b_pe       # [256]
    b_proj_eff = b_proj + w_proj_eff @ kappa

    dwdiag = np.zeros((2, 128, 25 * 128), np.float32)
    for ct in range(2):
        for tap in range(25):
            idx = np.arange(128)
            dwdiag[ct, idx, tap * 128 + idx] = wpe_eff[ct * 128 + idx, tap]

    common = {
        "wqkT": np.ascontiguousarray(w_qk_eff.T).astype(BF16NP),

        "wvT": np.ascontiguousarray(w_v_eff.T).astype(BF16NP),
        "wprojT": np.ascontiguousarray(w_proj_eff.T).astype(BF16NP),
        "bias": np.stack([b_qk_eff[0:128], b_qk_eff[128:256],
                          b_qk_eff[256:384], b_qk_eff[384:512],
                          b_proj_eff[0:128], b_proj_eff[128:256],
                          np.zeros(128, np.float32), np.zeros(128, np.float32)],
                         axis=1).astype(np.float32),
        "dwdiag": dwdiag.astype(BF16NP),
        "ones": np.ones((128, 512), BF16NP),
    }
    in_maps = []
    for i in range(8):
        m = dict(common)
        m["x"] = np.ascontiguousarray(x[i].reshape(C, NTOK)).astype(BF16NP)
        in_maps.append(m)
    return in_maps


def kernel(**inputs):
    nc = _get_nc()
    in_maps = _prep_host(inputs)
    res = run_bass_kernel_spmd(nc, in_maps, core_ids=list(range(8)))
    outs = [res.results[i]["out"].reshape(C, HH, WW) for i in range(8)]
    return np.stack(outs, 0).astype(np.float32)


if __name__ == "__main__":
    rng = np.random.default_rng(0)
    fake = {
        "x": rng.standard_normal((8, C, HH, WW), np.float32),
        "w_qk": rng.standard_normal((2 * C, C), np.float32) * 0.05,
        "s_qk": np.ones(2 * C, np.float32),
        "b_qk": rng.standard_normal(2 * C).astype(np.float32) * 0.01,
        "w_v": rng.standard_normal((C, C), np.float32) * 0.05,
        "s_v": np.ones(C, np.float32),
        "b_v": rng.standard_normal(C).astype(np.float32) * 0.01,
        "w_pe": rng.standard_normal((C, 1, 5, 5), np.float32) * 0.05,
        "s_pe": np.ones(C, np.float32),
        "b_pe": rng.standard_normal(C).astype(np.float32) * 0.01,
        "w_proj": rng.standard_normal((C, C), np.float32) * 0.05,
        "s_proj": np.ones(C, np.float32),
        "b_proj": rng.standard_normal(C).astype(np.float32) * 0.01,
    }
    out = kernel(**fake)
    print("out", out.shape, out.dtype, float(np.abs(out).mean()))



# revision 23
# speedup vs baseline: 1.6446x; 1.0019x over previous
"""Trainium2 Bass kernel for area-attention (YOLOv12 A2-style) module.

Raw-bass SPMD: 8 cores, 1 image each, no collectives. Engine split:
SP=DMA, PE=all matmuls (convs, dw-diag conv, attention with PE-array
row/col packing), ACT=softmax exp, DVE=evac/normalize/memset.
Software-pipelined per (area, head-group, q-tile) unit: exp(kc)
overlaps PV(kc-1)+scores(kc)+depthwise filler matmuls.
"""

import sys, os, functools

sys.path.insert(0, "/opt/trn_rl_repo")

import numpy as np
import ml_dtypes

import concourse.bass as bass
import concourse.mybir as mybir
from concourse.bass_utils import run_bass_kernel_spmd

BF16NP = ml_dtypes.bfloat16
F32 = mybir.dt.float32
BF = mybir.dt.bfloat16
I16 = mybir.dt.int16
ALU = mybir.AluOpType
SCH_K = 128.0 * float(np.log2(np.e))   # fold log2e into the DVE affine
SCH_B = 16248.636                      # mean-centered Schraudolph bias

C = 256
HH = 64
WW = 64
NTOK = HH * WW          # 4096
AREA = 4
NA = NTOK // AREA       # 1024
D = 32                  # head dim
SCALE = float(D) ** -0.5
PADW = WW + 4           # 68
PADH = HH + 4           # 68
NPAD = PADW * PADH      # 4624
AF = mybir.ActivationFunctionType

UNITS = [(a, hg, qt) for a in range(AREA) for hg in range(2) for qt in range(2)]
# dw output tile needed by proj of area a: (ct, 2a+qt)
DW_TILES = [(hg, 2 * a + qt) for a in range(AREA) for hg in range(2) for qt in range(2)]
N_IN_DMA = 12


def _build_nc():
    import contextlib
    nc = bass.Bass()

    x_d = nc.declare_dram_parameter("x", [C, NTOK], BF, isOutput=False)
    wqkT_d = nc.declare_dram_parameter("wqkT", [C, 2 * C], BF, isOutput=False)
    wvT_d = nc.declare_dram_parameter("wvT", [C, C], BF, isOutput=False)
    wprojT_d = nc.declare_dram_parameter("wprojT", [C, C], BF, isOutput=False)
    dwdiag_d = nc.declare_dram_parameter("dwdiag", [2, 128, 25 * 128], BF, isOutput=False)
    ones_d = nc.declare_dram_parameter("ones", [128, 512], BF, isOutput=False)
    bias_d = nc.declare_dram_parameter("bias", [128, 8], F32, isOutput=False)
    out_d = nc.declare_dram_parameter("out", [C, NTOK], F32, isOutput=True)

    ctx = contextlib.ExitStack()
    _names = [0]

    def sb(shape, dtype, nm=None):
        _names[0] += 1
        return ctx.enter_context(nc.sbuf_tensor(f"sb{_names[0]}", shape, dtype))[:, :]

    def ps(shape):
        _names[0] += 1
        return ctx.enter_context(nc.psum_tensor(f"ps{_names[0]}", shape, F32))[:, :]

    x_sb = [sb([128, NTOK], BF) for _ in range(2)]
    qk_sb = [sb([128, NTOK], BF) for _ in range(4)]
    vT_sb = sb([128, 32 * 256], BF)
    vpad_sb = [sb([128, NPAD], BF) for _ in range(2)]
    pp_sb = [sb([128, NTOK], BF) for _ in range(2)]
    oT_sb = [sb([128, NTOK], BF) for _ in range(2)]
    z_sb = [sb([128, NTOK], BF) for _ in range(2)]
    wqkT_sb = [sb([128, 2 * C], BF) for _ in range(2)]
    wvT_sb = [sb([128, C], BF) for _ in range(2)]
    wprojT_sb = [sb([128, C], BF) for _ in range(2)]
    dw_sb = [sb([128, 25 * 128], BF) for _ in range(2)]
    ones_sb = sb([128, 512], BF)
    bias_sb = sb([128, 8], F32)
    rcinv_sb = sb([128, 512], BF)
    es_sb = [sb([128, 1536], BF) for _ in range(2)]   # heads 0-2 (ACT exp)
    esd_sb = [sb([128, 512], BF) for _ in range(2)]   # head 3 (DVE Schraudolph)
    ucp_sb = sb([128, 512], F32)
    out_sb = [sb([128, 512], F32) for _ in range(16)]

    sc_ps = ps([128, 2048])   # 4 banks
    u_ps = ps([128, 512])
    r_ps = ps([128, 512])
    rb_ps = r_ps  # shared bank: recip reads r before bcast overwrites (sem-ordered)
    conv_ps = [ps([128, 512]) for _ in range(2)]

    vpad3 = [vp.rearrange("p (h w) -> p h w", h=PADH) for vp in vpad_sb]

    marks = {}  # name -> counter value (producer engine count at completion)

    # conv-bank users in PE order: list of keys; user i uses bank i%2 and
    # must WAR-wait on evac of user i-2 (marks['ev_'+key]).
    conv_users = []
    for m in range(4):
        for n in range(8):
            conv_users.append(f"qk_{m}_{n}")
    for t in range(32):
        conv_users.append(f"vt_{t}")
    for m in range(2):
        for n in range(8):
            conv_users.append(f"v_{m}_{n}")
    # dw tile u interleaved within unit u; proj tiles after each area
    seq = []
    for u, (a, hg, qt) in enumerate(UNITS):
        seq.append(f"dw_{u}")
        if hg == 1 and qt == 1:
            for nt in range(2):
                for m in range(2):
                    seq.append(f"pj_{2 * a + nt}_{m}")
    conv_users.extend(seq)
    user_bank = {k: i % 2 for i, k in enumerate(conv_users)}
    user_prev = {k: (conv_users[i - 2] if i >= 2 else None)
                 for i, k in enumerate(conv_users)}

    class Eng:
        """Counts instructions; in real mode also emits via `fns`."""

        def __init__(self, name, sem_name):
            self.name = name
            self.sem_name = sem_name
            self.n = 0

        def bump(self, binst, real, sems):
            self.n += 1
            if real:
                binst.then_inc(sems[self.sem_name], 1)

    def program(real, engines, sems):
        pe, act, dve, sp = engines["pe"], engines["act"], engines["dve"], engines["sp"]

        def w(eng_handle, sem_name, val):
            # standalone wait_ge; no count
            if real and val is not None and val > 0:
                eng_handle.wait_ge(sems[sem_name], val)

        def mark(name, eng):
            if not real:
                marks[name] = eng.n

        def get(name):
            return marks.get(name, 0)

        # ---------------- SP ----------------
        def sp_prog(h):
            n_dma = 0
            if real:
                for i in range(2):
                    h.dma_start(out=wqkT_sb[i], in_=wqkT_d[i * 128:(i + 1) * 128, :]).then_inc(sems["dma_i"], 16)
                    h.dma_start(out=wvT_sb[i], in_=wvT_d[i * 128:(i + 1) * 128, :]).then_inc(sems["dma_i"], 16)
                    h.dma_start(out=wprojT_sb[i], in_=wprojT_d[i * 128:(i + 1) * 128, :]).then_inc(sems["dma_i"], 16)
                    h.dma_start(out=dw_sb[i], in_=dwdiag_d[i, :, :]).then_inc(sems["dma_i"], 16)
                h.dma_start(out=ones_sb, in_=ones_d[:, :]).then_inc(sems["dma_i"], 16)
                h.dma_start(out=bias_sb, in_=bias_d[:, :]).then_inc(sems["dma_i"], 16)
                for i in range(2):
                    h.dma_start(out=x_sb[i], in_=x_d[i * 128:(i + 1) * 128, :]).then_inc(sems["dma_i"], 16)
            n_dma = 12
            assert n_dma == N_IN_DMA
            # out DMAs
            for k in range(8):            # token tile n
                for m in range(2):
                    idx = k * 2 + m
                    if real:
                        h.wait_ge(sems["dve_s"], marks[f"po_{k}_{m}"])
                        h.dma_start(out=out_d[m * 128:(m + 1) * 128, k * 512:(k + 1) * 512],
                                    in_=out_sb[idx]).then_inc(sems["dma_o"], 16)
            if real:
                h.wait_ge(sems["dma_o"], 16 * 16)

        # ---------------- PE ----------------
        def pe_prog(h):
            def mm(out, lhsT, rhs, start, stop, tp=None, inc=False):
                if real:
                    i = nc.tensor.matmul(out, lhsT, rhs, start=start, stop=stop,
                                         tile_position=tp, skip_group_check=True)
                    if inc:
                        pe.bump(i, real, sems)
                elif inc:
                    pe.n += 1

            def conv_war(key):
                prev = user_prev[key]
                if prev is not None:
                    # vt/v evacs run on ACT, everything else on DVE
                    sem = "act_s" if prev.startswith(("vt_", "v_")) else "dve_s"
                    w(h, sem, get(f"ev_{prev}"))

            w(h, "dma_i", N_IN_DMA * 16)
            # qk GEMM
            for m in range(4):
                for n in range(8):
                    key = f"qk_{m}_{n}"
                    b = user_bank[key]
                    conv_war(key)
                    mm(conv_ps[b], wqkT_sb[0][:, m * 128:(m + 1) * 128],
                       x_sb[0][:, n * 512:(n + 1) * 512], True, False)
                    mm(conv_ps[b], wqkT_sb[1][:, m * 128:(m + 1) * 128],
                       x_sb[1][:, n * 512:(n + 1) * 512], False, True, inc=True)
                    mark(key, pe)
            # vT GEMM
            for t in range(32):
                key = f"vt_{t}"
                b = user_bank[key]
                conv_war(key)
                mm(conv_ps[b][:, 0:256], x_sb[0][:, t * 128:(t + 1) * 128], wvT_sb[0], True, False)
                mm(conv_ps[b][:, 0:256], x_sb[1][:, t * 128:(t + 1) * 128], wvT_sb[1], False, True, inc=True)
                mark(key, pe)
            # v GEMM
            for m in range(2):
                for n in range(8):
                    key = f"v_{m}_{n}"
                    b = user_bank[key]
                    conv_war(key)
                    mm(conv_ps[b], wvT_sb[0][:, m * 128:(m + 1) * 128],
                       x_sb[0][:, n * 512:(n + 1) * 512], True, False)
                    mm(conv_ps[b], wvT_sb[1][:, m * 128:(m + 1) * 128],
                       x_sb[1][:, n * 512:(n + 1) * 512], False, True, inc=True)
                    mark(key, pe)
            # wait all startup evacs (qk on DVE; vT/vpad on ACT)
            w(h, "dve_s", get("qk_evac_done"))
            w(h, "act_s", get("startup_evac"))

            def scores_g(u2, kc):
                a2, hg2, qt2 = UNITS[u2]
                qb2 = a2 * NA + qt2 * 512
                kb = a2 * NA + kc * 128
                for j in range(4):
                    mm(sc_ps[:, j * 512:(j + 1) * 512],
                       qk_sb[2 + hg2][32 * j:32 * j + 32, kb:kb + 128],
                       qk_sb[hg2][32 * j:32 * j + 32, qb2:qb2 + 512],
                       True, True, tp=(32 * j, 0), inc=(j == 3))
                mark(f"grp_{u2}_{kc}", pe)

            for u, (a, hg, qt) in enumerate(UNITS):
                qb = a * NA + qt * 512

                def scores(kc):
                    scores_g(u, kc)

                def esj_of(kc, j):
                    if j < 3:
                        return es_sb[kc % 2][:, j * 512:(j + 1) * 512]
                    return esd_sb[kc % 2][:, :]

                def pv_part(kc, js, inc_last):
                    tvt = a * 8 + kc
                    for j in js:
                        mm(u_ps[32 * j:32 * j + 32, :],
                           vT_sb[:, tvt * 256 + hg * 128 + 32 * j:
                                 tvt * 256 + hg * 128 + 32 * j + 32],
                           esj_of(kc, j), kc == 0, kc == 7, tp=(0, 32 * j))
                    for j in js:
                        mm(r_ps[32 * j:32 * j + 1, :], ones_sb[:, 0:1],
                           esj_of(kc, j), kc == 0, kc == 7, tp=(0, 32 * j),
                           inc=(inc_last and j == js[-1]))

                # dw filler chunks for dw tile u
                ct, dn = DW_TILES[u]
                dwkey = f"dw_{u}"
                dwb = user_bank[dwkey]

                def dw_chunk(ci):
                    taps = range((25 * ci) // 8, (25 * (ci + 1)) // 8)
                    for tap in taps:
                        if tap == 0:
                            conv_war(dwkey)
                        dy, dx = divmod(tap, 5)
                        mm(conv_ps[dwb], dw_sb[ct][:, tap * 128:(tap + 1) * 128],
                           vpad3[ct][:, 8 * dn + dy:8 * dn + dy + 8, dx:dx + WW],
                           tap == 0, tap == 24, inc=(tap == 24))
                    if taps and max(taps) == 24:
                        mark(dwkey, pe)

                # unit prologue: WAR on U/r/rb banks vs previous unit's DVE
                # reads. scores(0) of unit 0 issues here; later units get
                # their scores(0) hoisted to the end of the previous unit.
                if u > 0:
                    w(h, "dve_s", get(f"unit_dve_{u - 1}"))
                else:
                    scores(0)
                for kc in range(1, 9):
                    dw_chunk(kc - 1)
                    w(h, "act_s", get(f"exp_{u}_{kc - 1}"))
                    w(h, "dve_s", get(f"expd_{u}_{kc - 1}"))
                    # scores(kc) FIRST: it has the same dependency as pv
                    # (exp/expd(kc-1) done frees the sc banks) and is the only
                    # thing ACT waits on — issuing it before pv lets the next
                    # exp start ~1.3us earlier; pv/r hide under the exp window.
                    if kc < 8:
                        scores(kc)
                    pv_part(kc - 1, [0, 1, 2, 3], True)
                mark(f"unitpv_{u}", pe)
                # Hoist the NEXT unit's first scores ahead of the reciprocal
                # wait so ACT computes exp(u+1, 0) during the recip+rb+mul
                # window (sc banks are free: exp/expd(u,7) completed before
                # pv(7)).
                if u + 1 < len(UNITS):
                    scores_g(u + 1, 0)
                # rb broadcast (needs recip on DVE)
                w(h, "dve_s", get(f"recip_{u}"))
                for j in range(4):
                    mm(rb_ps[32 * j:32 * j + 32, :],
                       ones_sb[32 * j:32 * j + 1, 0:32],
                       rcinv_sb[32 * j:32 * j + 1, :],
                       True, True, tp=(32 * j, 32 * j), inc=(j == 3))
                mark(f"rb_{u}", pe)

                if hg == 1 and qt == 1:
                    # proj for area a
                    w(h, "dve_s", get(f"z_{a}"))
                    for nt in range(2):
                        k = 2 * a + nt
                        for m in range(2):
                            key = f"pj_{k}_{m}"
                            b = user_bank[key]
                            conv_war(key)
                            mm(conv_ps[b], wprojT_sb[0][:, m * 128:(m + 1) * 128],
                               z_sb[0][:, k * 512:(k + 1) * 512], True, False)
                            mm(conv_ps[b], wprojT_sb[1][:, m * 128:(m + 1) * 128],
                               z_sb[1][:, k * 512:(k + 1) * 512], False, True, inc=True)
                            mark(key, pe)

        # ---------------- ACT ----------------
        def act_prog(h):
            def ex(out, in_):
                if real:
                    i = nc.scalar.activation(out, in_, AF.Exp)
                    act.bump(i, real, sems)
                else:
                    act.n += 1

            def cop(out, in_):
                if real:
                    i = nc.scalar.activation(out, in_, AF.Copy)
                    act.bump(i, real, sems)
                else:
                    act.n += 1

            # vT / v->vpad evacs (frees DVE for qk evacs; order matches PE's
            # GEMM order qk->vt->v, so no wait cycles)
            for t in range(32):
                key = f"vt_{t}"
                w(h, "pe_s", get(key))
                cop(vT_sb[:, t * 256:(t + 1) * 256],
                    conv_ps[user_bank[key]][:, 0:256])
                mark(f"ev_{key}", act)
            w(h, "dve_s", get("memsets_done"))
            for m in range(2):
                for n in range(8):
                    key = f"v_{m}_{n}"
                    w(h, "pe_s", get(key))
                    cop(vpad3[m][:, 2 + 8 * n:2 + 8 * n + 8, 2:2 + WW],
                        conv_ps[user_bank[key]].rearrange("p (r w) -> p r w", r=8))
                    mark(f"ev_{key}", act)
            mark("startup_evac", act)

            for u, (a, hg, qt) in enumerate(UNITS):
                for kc in range(8):
                    w(h, "pe_s", get(f"grp_{u}_{kc}"))
                    ex(es_sb[kc % 2], sc_ps[:, 0:1536])
                    mark(f"exp_{u}_{kc}", act)

        # ---------------- DVE ----------------
        def dve_prog(h):
            def selfwait():
                if real:
                    h.wait_ge(sems["dve_s"], dve.n)

            def op(fn, *args, **kw):
                if real:
                    i = fn(*args, **kw)
                    dve.bump(i, real, sems)
                else:
                    dve.n += 1

            op(nc.vector.memset, vpad_sb[0], 0.0)
            op(nc.vector.memset, vpad_sb[1], 0.0)
            op(nc.vector.memset, r_ps, 1.0)
            mark("memsets_done", dve)

            # qk evac with bias add (kills the rank-1 bias matmuls)
            for m in range(4):
                for n in range(8):
                    key = f"qk_{m}_{n}"
                    w(h, "pe_s", get(key))
                    op(nc.vector.tensor_scalar,
                       qk_sb[m][:, n * 512:(n + 1) * 512],
                       conv_ps[user_bank[key]],
                       bias_sb[:, m:m + 1], None, ALU.add)
                    mark(f"ev_{key}", dve)
            mark("qk_evac_done", dve)

            for u, (a, hg, qt) in enumerate(UNITS):
                qb = a * NA + qt * 512
                # head-3 exp via Schraudolph bit trick (single-bank PSUM read,
                # whole-tensor bitcast)
                for kc in range(8):
                    w(h, "pe_s", get(f"grp_{u}_{kc}"))
                    op(nc.vector.tensor_scalar,
                       esd_sb[kc % 2].bitcast(I16), sc_ps[:, 1536:2048],
                       SCH_K, SCH_B, ALU.mult, ALU.add)
                    mark(f"expd_{u}_{kc}", dve)
                w(h, "pe_s", get(f"unitpv_{u}"))
                selfwait()
                if real:
                    lp = nc.allow_low_precision("softmax denom to bf16")
                    lp.__enter__()
                op(nc.vector.reciprocal, rcinv_sb, r_ps)
                if real:
                    lp.__exit__(None, None, None)
                mark(f"recip_{u}", dve)
                op(nc.vector.tensor_copy, ucp_sb, u_ps)
                w(h, "pe_s", get(f"rb_{u}"))
                selfwait()
                op(nc.vector.tensor_mul,
                   oT_sb[hg][:, qb:qb + 512], ucp_sb, rb_ps)
                mark(f"unit_dve_{u}", dve)

                # dw evac for tile u
                ct, dn = DW_TILES[u]
                dwkey = f"dw_{u}"
                w(h, "pe_s", get(dwkey))
                op(nc.vector.tensor_copy,
                   pp_sb[ct][:, dn * 512:(dn + 1) * 512], conv_ps[user_bank[dwkey]])
                mark(f"ev_{dwkey}", dve)

                if hg == 1 and qt == 1:
                    # z = o + pp for area a
                    selfwait()
                    for cti in range(2):
                        op(nc.vector.tensor_add,
                           z_sb[cti][:, a * NA:(a + 1) * NA],
                           oT_sb[cti][:, a * NA:(a + 1) * NA],
                           pp_sb[cti][:, a * NA:(a + 1) * NA])
                    mark(f"z_{a}", dve)
                    # proj evacs
                    for nt in range(2):
                        k = 2 * a + nt
                        for m in range(2):
                            key = f"pj_{k}_{m}"
                            idx = k * 2 + m
                            w(h, "pe_s", get(key))
                            op(nc.vector.tensor_scalar, out_sb[idx],
                               conv_ps[user_bank[key]],
                               bias_sb[:, 4 + m:5 + m], None, ALU.add)
                            mark(f"ev_{key}", dve)
                            mark(f"po_{k}_{m}", dve)

        if real:
            with nc.Block() as block, \
                 nc.semaphore("dma_i") as s_dma_i, \
                 nc.semaphore("dma_o") as s_dma_o, \
                 nc.semaphore("pe_s") as s_pe, \
                 nc.semaphore("act_s") as s_act, \
                 nc.semaphore("dve_s") as s_dve:
                sems.update({"dma_i": s_dma_i, "dma_o": s_dma_o,
                             "pe_s": s_pe, "act_s": s_act, "dve_s": s_dve})

                @block.sync
                def _(sync):
                    sp_prog(sync)

                @block.tensor
                def _(tensor):
                    pe_prog(tensor)

                @block.scalar
                def _(scalar):
                    act_prog(scalar)

                @block.vector
                def _(vector):
                    dve_prog(vector)
        else:
            class H:  # dry handle
                def wait_ge(self, *a, **k):
                    pass

                def dma_start(self, *a, **k):
                    class R:
                        def then_inc(self, *a, **k):
                            return self
                    return R()
            hh = H()
            sp_prog(hh)
            pe_prog(hh)
            act_prog(hh)
            dve_prog(hh)

    engines = {"pe": Eng("pe", "pe_s"), "act": Eng("act", "act_s"),
               "dve": Eng("dve", "dve_s"), "sp": Eng("sp", "dma_i")}
    sems = {}
    program(False, engines, sems)          # dry: fill marks
    engines = {"pe": Eng("pe", "pe_s"), "act": Eng("act", "act_s"),
               "dve": Eng("dve", "dve_s"), "sp": Eng("sp", "dma_i")}
    program(True, engines, sems)           # real emission
    return nc


@functools.lru_cache(maxsize=1)
def _get_nc():
    return _build_nc()


def _prep_host(inputs):
    x = np.asarray(inputs["x"], np.float32)            # [8, 256, 64, 64]
    w_qk = np.asarray(inputs["w_qk"], np.float32)      # [512, 256]
    s_qk = np.asarray(inputs["s_qk"], np.float32)
    b_qk = np.asarray(inputs["b_qk"], np.float32)
    w_v = np.asarray(inputs["w_v"], np.float32)
    s_v = np.asarray(inputs["s_v"], np.float32)
    b_v = np.asarray(inputs["b_v"], np.float32)
    w_pe = np.asarray(inputs["w_pe"], np.float32)      # [256, 1, 5, 5]
    s_pe = np.asarray(inputs["s_pe"], np.float32)
    b_pe = np.asarray(inputs["b_pe"], np.float32)
    w_proj = np.asarray(inputs["w_proj"], np.float32)
    s_proj = np.asarray(inputs["s_proj"], np.float32)
    b_proj = np.asarray(inputs["b_proj"], np.float32)

    # fold BN scales into weights; fold 1/sqrt(d) into q weights+bias
    w_qk_eff = w_qk * s_qk[:, None]
    b_qk_eff = b_qk * s_qk  # BN affine: y = s*(Wx) + b ... b is already the bias
    # NB: reference _conv1x1 computes  y = (Wx)*s + b, so bias is NOT scaled by s.
    b_qk_eff = b_qk.copy()
    w_qk_eff[:C] *= SCALE
    b_qk_eff[:C] *= SCALE

    w_v_eff = w_v * s_v[:, None]
    w_proj_eff = w_proj * s_proj[:, None]

    wpe = w_pe.reshape(C, 25)                          # [c, tap]
    wpe_eff = wpe * s_pe[:, None]

    # constants folded through attention/depthwise into proj bias:
    # o gets +b_v exactly (softmax rows sum to 1);
    # pp = s_pe*dw(v_nb) + s_pe*b_v*sum_taps(w_pe) + b_pe
    kappa = b_v + s_pe * b_v * wpe.sum(1) + b_pe       # [256]
    b_proj_eff = b_proj + w_proj_eff @ kappa

    dwdiag = np.zeros((2, 128, 25 * 128), np.float32)
    for ct in range(2):
        for tap in range(25):
            idx = np.arange(128)
            dwdiag[ct, idx, tap * 128 + idx] = wpe_eff[ct * 128 + idx, tap]

    common = {
        "wqkT": np.ascontiguousarray(w_qk_eff.T).astype(BF16NP),

        "wvT": np.ascontiguousarray(w_v_eff.T).astype(BF16NP),
        "wprojT": np.ascontiguousarray(w_proj_eff.T).astype(BF16NP),
        "bias": np.stack([b_qk_eff[0:128], b_qk_eff[128:256],
                          b_qk_eff[256:384], b_qk_eff[384:512],
                          b_proj_eff[0:128], b_proj_eff[128:256],
                          np.zeros(128, np.float32), np.zeros(128, np.float32)],
                         axis=1).astype(np.float32),
        "dwdiag": dwdiag.astype(BF16NP),
        "ones": np.ones((128, 512), BF16NP),
    }
    in_maps = []
    for i in range(8):
        m = dict(common)
        m["x"] = np.ascontiguousarray(x[i].reshape(C, NTOK)).astype(BF16NP)
        in_maps.append(m)
    return in_maps


def kernel(**inputs):
    nc = _get_nc()
    in_maps = _prep_host(inputs)
    res = run_bass_kernel_spmd(nc, in_maps, core_ids=list(range(8)))
    outs = [res.results[i]["out"].reshape(C, HH, WW) for i in range(8)]
    return np.stack(outs, 0).astype(np.float32)


if __name__ == "__main__":
    rng = np.random.default_rng(0)
    fake = {
        "x": rng.standard_normal((8, C, HH, WW), np.float32),
        "w_qk": rng.standard_normal((2 * C, C), np.float32) * 0.05,
        "s_qk": np.ones(2 * C, np.float32),
        "b_qk": rng.standard_normal(2 * C).astype(np.float32) * 0.01,
        "w_v": rng.standard_normal((C, C), np.float32) * 0.05,
        "s_v": np.ones(C, np.float32),
        "b_v": rng.standard_normal(C).astype(np.float32) * 0.01,
        "w_pe": rng.standard_normal((C, 1, 5, 5), np.float32) * 0.05,
        "s_pe": np.ones(C, np.float32),
        "b_pe": rng.standard_normal(C).astype(np.float32) * 0.01,
        "w_proj": rng.standard_normal((C, C), np.float32) * 0.05,
        "s_proj": np.ones(C, np.float32),
        "b_proj": rng.standard_normal(C).astype(np.float32) * 0.01,
    }
    out = kernel(**fake)
    print("out", out.shape, out.dtype, float(np.abs(out).mean()))

